# revision 48
# speedup vs baseline: 1.2059x; 1.0080x over previous
"""Trainium2 Bass kernel for a Transformer-XL (MemTransformerLM) layer.

Sharding over 8 cores: core c = (b = c//4, head-group g = c%4 of 4 heads).
Each core computes its 4 heads' attention for its batch, a partial
attn_out = vec @ W_o[:, heads].T, then a ReduceScatter(+) over the quad
[[0..3],[4..7]] scatters query rows -> each core does LN1+FF+LN2 on its
256 rows. Host reassembles [1024, 2, 1024].

rel_shift: B = q_tilde @ rk^T is written per head to DRAM (f8e4, row
stride 2176) in one 4-query-tile batch per half; BD[i,j] = B[i, j-i+1023]
is read back with a batched oblique AP (4 query tiles per DMA), converted
f8->f32 on GPSIMD, and PE-transpose-accumulated into the AC^T PSUM group.
Masking comes free from -240 pad columns (exp underflows to 0); softmax
denominator from a ones-column appended to v.

W1 (and the first 8 W2 row-tiles) are prefetched during the attention
phase into the SBUF slots freed by cat/r/pw, so the FF phase starts
immediately after the ReduceScatter. LN1's affine is folded into W1/b1
host-side (exact); the residual copy of out1 gets the affine lazily on
GPSIMD off the critical path. Both LayerNorms use fused one-pass
sum/var accumulation (scalar_tensor_tensor accum_out + Act Square
accum_out) spread across DVE/Act/Pool.
"""
import functools
import numpy as np

QLEN, MLEN, BSZ = 1024, 1024, 2
KLEN = QLEN + MLEN
D, H, DH, DI = 1024, 16, 64, 4096
HPG = 4                      # heads per group (per core)
HD_G = HPG * DH              # 256
N_CORES = 8
SCALE = 1.0 / (DH ** 0.5)
NEG = -1e30
BW = 2176                    # padded DRAM width for B (>= 2175)
NQT = QLEN // 128            # 8 query tiles of 128
NKT = KLEN // 128            # 16 key tiles of 128
NDC = D // 128               # 8 d-chunks
NMI = DI // 128              # 32 inner tiles
ROWS = QLEN // 4             # 256 rows per core after RS


@functools.lru_cache(maxsize=2)
def _build(single_sim=False):
    import concourse.bacc as bacc
    import concourse.mybir as mybir
    import concourse.tile as tile
    from concourse import masks
    import bass_rust

    F32 = mybir.dt.float32
    BF16 = mybir.dt.bfloat16
    F8 = mybir.dt.float8e4
    AF = mybir.ActivationFunctionType
    ALU = mybir.AluOpType

    nc = bacc.Bacc("TRN2", target_bir_lowering=False, debug=False,
                   num_devices=N_CORES)

    def din(name, shape, dt=F32):
        return nc.dram_tensor(name, shape, dt, kind="ExternalInput")

    cat_fm = din("cat_fm", [D, KLEN], BF16)   # [d, mems||w tokens], this b
    r_fm = din("r_fm", [D, KLEN], F8)         # r transposed
    wpk = din("wpk", [D, 3 * HD_G], F8)       # [wkT | wqT | wrT]
    wvk = din("wvk", [D, HD_G], BF16)         # wvT
    biases = din("biases", [128, 8])          # raw rwb/rrb + pre-scaled
    woT = din("woT", [HD_G, D], BF16)         # W_o^T rows for group
    w1T = din("w1T", [D, DI], BF16)           # g1 pre-folded in
    b1c = din("b1c", [128, NMI])              # b1 (+W1@ln1_b) packed col-wise
    w2T = din("w2T", [DI, D], BF16)
    gbr = din("gbr", [5 * 128, D], BF16)      # rows: ffb2, g1, b1, g2, b2
    wres = din("wres", [ROWS, D], BF16)       # w rows for residual

    Bh = [nc.dram_tensor(f"Bh{h}", [QLEN * BW], F8) for h in range(HPG)]
    if single_sim:
        attn_part = nc.dram_tensor("attn_part", [QLEN, D], BF16,
                                   kind="ExternalOutput")
    else:
        attn_part = nc.dram_tensor("attn_part", [QLEN, D], BF16)
    rs_out = nc.dram_tensor("rs_out", [ROWS, D], BF16)
    y = nc.dram_tensor("y", [ROWS, D], F32, kind="ExternalOutput")

    def obl4(h, half, ktb):
        # batched oblique: BD tiles [128 q, 4 qtiles, 512 keys] at
        # (qt = 4*half + qti, kt = 4*ktb)
        off = 1023 + 512 * half * (BW - 1) + 512 * ktb
        return bass_rust.AP(tensor=Bh[h].ap().tensor, offset=off,
                            ap=[[BW - 1, 128], [128 * (BW - 1), 4], [1, 512]])

    def bwrite4(h, half, c0):
        # B row block [128, 4 qtiles, BW-? cols] at rows 512*half, col c0
        off = 512 * half * BW + c0
        return bass_rust.AP(tensor=Bh[h].ap().tensor, offset=off,
                            ap=[[BW, 128], [128 * BW, 4], [1, KLEN - c0]])

    def bpad(h):
        # all pad columns of head h as one flat write
        off = 2048
        return bass_rust.AP(tensor=Bh[h].ap().tensor, offset=off,
                            ap=[[128 * BW, NQT], [BW, 128], [1, BW - 2048]])

    with tile.TileContext(nc) as tc:
        with tc.tile_pool(name="const", bufs=1) as cpool, \
             tc.tile_pool(name="slots", bufs=1) as spool, \
             tc.tile_pool(name="work", bufs=2) as wpool, \
             tc.tile_pool(name="psA", bufs=3, space="PSUM") as psA, \
             tc.tile_pool(name="psB", bufs=3, space="PSUM") as psB, \
             tc.tile_pool(name="psV", bufs=1, space="PSUM") as psV, \
             tc.tile_pool(name="psT", bufs=1, space="PSUM") as psT:

            # ---------------- global constants ----------------
            identb = cpool.tile([128, 128], BF16, tag="identb")
            masks.make_identity(nc, identb[:])
            identr = cpool.tile([128, 128], mybir.dt.float32r, tag="identr")
            nc.scalar.activation(identr[:], identb[:], AF.Copy)
            bias_t = cpool.tile([128, 8], F32, tag="bias")
            nc.scalar.dma_start(out=bias_t[:], in_=biases[:])
            b1c_t = cpool.tile([128, NMI], F32, tag="b1c")
            nc.scalar.dma_start(out=b1c_t[:], in_=b1c[:])
            # out1 lives across scopes: normalized (t1) + affined copies
            out1n = [cpool.tile([128, D], BF16, tag=f"o1n{t}", name=f"o1n{t}")
                     for t in range(ROWS // 128)]
            out1a = [cpool.tile([128, D], BF16, tag=f"o1a{t}", name=f"o1a{t}")
                     for t in range(ROWS // 128)]

            wres_t = [cpool.tile([128, D], BF16, tag=f"wres{t}",
                                 name=f"wres{t}") for t in range(ROWS // 128)]
            onesr = cpool.tile([1, 128], BF16, tag="onesr")
            nc.vector.memset(onesr[:], 1.0)
            # pad cols: large-negative f8 so exp() underflows to 0
            zpad = cpool.tile([128, (BW - 2048) * NQT], F8, tag="zpad")
            nc.vector.memset(zpad[:], -240.0)
            for h in range(HPG):
                nc.scalar.dma_start(out=bpad(h), in_=zpad[:])

            # ------------- reusable big slots (outer pool) -------------
            # catA/rA: 4 tags x [128, 2, 2048] bf16 each (1MB) ->
            # later reused for W1 blocks. pwA: 4 tags x [128, 2, 1024]
            # -> later reused for the first 8 W2 row-tiles.
            catB, rA, pwA, pwV = [], [], [], []
            for k in range(NDC):
                t = spool.tile([128, KLEN], BF16, tag=f"catB{k}",
                               name=f"catB{k}")
                nc.sync.dma_start(out=t[:],
                                  in_=cat_fm[128 * k:128 * k + 128, :])
                catB.append(t)
                if k % 2 == 1:
                    j = k // 2
                    t = spool.tile([128, 2, 3 * HD_G], F8, tag=f"pwA{j}",
                                   name=f"pwA{j}",
                                   padded_shape=[128, 2, 2 * D])
                    nc.sync.dma_start(
                        out=t[:], in_=wpk[256 * j:256 * j + 256, :].rearrange(
                            "(a b) c -> b a c", a=2, b=128))
                    pwA.append(t)
                    t = spool.tile([128, 2, HD_G], BF16, tag=f"pwV{j}",
                                   name=f"pwV{j}")
                    nc.sync.dma_start(
                        out=t[:], in_=wvk[256 * j:256 * j + 256, :].rearrange(
                            "(a b) c -> b a c", a=2, b=128))
                    pwV.append(t)
            for j in range(4):
                t = spool.tile([128, 2, KLEN], F8, tag=f"rA{j}",
                               name=f"rA{j}", padded_shape=[128, 2, 2 * KLEN])
                nc.sync.dma_start(
                    out=t[:], in_=r_fm[256 * j:256 * j + 256, :].rearrange(
                        "(a b) c -> b a c", a=2, b=128))
                rA.append(t)

            def cat_sl(k):
                return catB[k][:]

            def r_sl(k):
                return rA[k // 2][:, k % 2, :]

            def pw_sl(k, which, m):
                base = {"wkT": 0, "wqT": 1, "wrT": 2}[which] * HD_G
                return pwA[k // 2][:, k % 2, base + 128 * m:base + 128 * m + 128]

            def pw_v(k):
                return pwV[k // 2][:, k % 2, :]

            # ================ attention scope ================
            with tc.tile_pool(name="attn", bufs=1) as apool, \
                 tc.tile_pool(name="prob", bufs=2) as ppool:

                woT_t = apool.tile([128, 2, D], BF16, tag="woT", name="woT_t")

                k_fm, rk_fm, qh_fm, qt_fm = [], [], [], []
                for m in range(2):
                    k_fm.append(apool.tile([128, KLEN], BF16, tag=f"kfm{m}",
                                           name=f"kfm{m}"))
                    rk_fm.append(apool.tile([128, KLEN], F8, tag=f"rkfm{m}",
                                            name=f"rkfm{m}"))
                    qh_fm.append(apool.tile([128, QLEN], BF16, tag=f"qhfm{m}",
                                            name=f"qhfm{m}"))
                    qt_fm.append(apool.tile([128, QLEN], F8, tag=f"qtfm{m}",
                                            name=f"qtfm{m}"))
                for m in range(2):
                    for n in range(KLEN // 512):
                        ps = psA.tile([128, 512], F32, tag="psA", name="psk")
                        for k in range(NDC):
                            nc.tensor.matmul(
                                ps[:], pw_sl(k, "wkT", m),
                                cat_sl(k)[:, 512 * n:512 * n + 512],
                                start=(k == 0), stop=(k == NDC - 1))
                        nc.scalar.activation(k_fm[m][:, 512 * n:512 * n + 512],
                                             ps[:], AF.Copy)
                    for n in range(QLEN // 512):
                        ps = psA.tile([128, 512], F32, tag="psA", name="psq")
                        for k in range(NDC):
                            nc.tensor.matmul(
                                ps[:], pw_sl(k, "wqT", m),
                                cat_sl(k)[:, MLEN + 512 * n:MLEN + 512 * n + 512],
                                start=(k == 0), stop=(k == NDC - 1))
                        # (q + bias) * SCALE on DVE, cast to bf16
                        nc.vector.tensor_scalar(
                            out=qh_fm[m][:, 512 * n:512 * n + 512], in0=ps[:],
                            scalar1=bias_t[:, m:m + 1], scalar2=SCALE,
                            op0=ALU.add, op1=ALU.mult)
                        nc.scalar.activation(
                            qt_fm[m][:, 512 * n:512 * n + 512], ps[:],
                            AF.Identity, scale=SCALE,
                            bias=bias_t[:, 6 + m:7 + m])
                # v token-major with interleaved ones cols: [128, 4, 65]
                v_tok = []
                for kt in range(NKT):
                    vt = apool.tile([128, HPG, 65], BF16, tag=f"vtok{kt}",
                                    name=f"vtok{kt}")
                    ps = psB.tile([128, HD_G], F32, tag="psB", name="psv")
                    for k in range(NDC):
                        nc.tensor.matmul(
                            ps[:], cat_sl(k)[:, 128 * kt:128 * kt + 128],
                            pw_v(k), start=(k == 0), stop=(k == NDC - 1))
                    nc.vector.memset(vt[:, :, 64:65], 1.0)
                    nc.scalar.activation(
                        vt[:, :, 0:64],
                        ps[:].rearrange("p (a b) -> p a b", a=HPG, b=64),
                        AF.Copy)
                    v_tok.append(vt)
                # rk projection
                for m in range(2):
                    for n in range(KLEN // 512):
                        ps = psA.tile([128, 512], F32, tag="psA", name="psr")
                        for k in range(NDC):
                            nc.tensor.matmul(
                                ps[:], pw_sl(k, "wrT", m),
                                r_sl(k)[:, 512 * n:512 * n + 512],
                                start=(k == 0), stop=(k == NDC - 1))
                        nc.scalar.activation(rk_fm[m][:, 512 * n:512 * n + 512],
                                             ps[:], AF.Copy)

                # ---- FF weight prefetch tiles (fill freed cat/r/pw slots).
                # DMAs are issued interleaved into the head loop below;
                # each DMA <= 512KB to bound queue head-of-line delay.
                w1blk = [[], []]          # [0][k]: [128,2048]; [1][j]: [128,2,2048]
                for k in range(NDC):
                    w1blk[0].append(spool.tile([128, KLEN], BF16,
                                               tag=f"catB{k}", name=f"w1b0_{k}"))
                for j in range(4):
                    w1blk[1].append(spool.tile([128, 2, KLEN], BF16,
                                               tag=f"rA{j}", name=f"w1b1_{j}"))
                w2pre = []
                for j in range(4):
                    w2pre.append(spool.tile([128, 2, D], BF16, tag=f"pwA{j}",
                                            name=f"w2pre{j}"))
                pf_dmas = []
                for k in range(NDC):
                    pf_dmas.append((w1blk[0][k][:],
                                    w1T[128 * k:128 * k + 128, 0:2048]))
                for j in range(4):
                    for j2 in range(2):
                        pf_dmas.append((
                            w1blk[1][j][:, j2, :],
                            w1T[256 * j + 128 * j2:256 * j + 128 * j2 + 128,
                                2048:4096]))
                for j in range(4):
                    for j2 in range(2):
                        pf_dmas.append((
                            w2pre[j][:, j2, :],
                            w2T[256 * j + 128 * j2:256 * j + 128 * j2 + 128, :]))
                for t in range(ROWS // 128):
                    pf_dmas.append((wres_t[t][:],
                                    wres[128 * t:128 * t + 128, :]))
                pf_dmas.insert(4, (woT_t[:], woT[:].rearrange(
                    "(a b) c -> b a c", a=2, b=128)))

                def w1_sl(mi, k):
                    mo = 128 * (mi % 16)
                    if mi < 16:
                        return w1blk[0][k][:, mo:mo + 128]
                    return w1blk[1][k // 2][:, k % 2, mo:mo + 128]

                # ---------------- P2: attention per head ----------------
                vecT_fm = {}
                for m in range(2):
                    for hf in range(2):
                        vecT_fm[(m, hf)] = apool.tile(
                            [128, QLEN // 2], BF16, tag=f"vecT{m}_{hf}",
                            name=f"vecT{m}_{hf}")
                pf_iter = iter(pf_dmas)

                def issue_pf(n):
                    for _ in range(n):
                        item = next(pf_iter, None)
                        if item is None:
                            return
                        dst, src = item
                        nc.sync.dma_start(out=dst, in_=src)

                for h in range(HPG):
                    m, p0 = h // 2, 64 * (h % 2)
                    qh_h = qh_fm[m][p0:p0 + 64, :]
                    qt_h = qt_fm[m][p0:p0 + 64, :]
                    k_h = k_fm[m][p0:p0 + 64, :]
                    rk_h = rk_fm[m][p0:p0 + 64, :]

                    # B = q_tilde @ rk^T -> DRAM f8 rows, one DMA per half.
                    # For qt<=3 the first 512 cols are never read back.
                    for half in range(2):
                        ct0 = 1 - half
                        bs = wpool.tile([128, 4, KLEN - 512 * ct0], F8,
                                        tag=f"bstage{half}", bufs=1, name="bs",
                                        padded_shape=[128, 4, KLEN - 512 * ct0])
                        for qti in range(4):
                            qt = 4 * half + qti
                            for ct in range(ct0, KLEN // 512):
                                ps = psB.tile([128, 512], F32, tag="psB",
                                              name="psb")
                                nc.tensor.matmul(
                                    ps[:], qt_h[:, 128 * qt:128 * qt + 128],
                                    rk_h[:, 512 * ct:512 * ct + 512],
                                    start=True, stop=True)
                                co = 512 * (ct - ct0)
                                if ct == 1:
                                    nc.scalar.activation(
                                        bs[:, qti, co:co + 512], ps[:], AF.Copy)
                                else:
                                    nc.vector.tensor_copy(
                                        bs[:, qti, co:co + 512], ps[:])
                        nc.sync.dma_start(out=bwrite4(h, half, 512 * ct0),
                                          in_=bs[:])

                    for qh2 in range(2):       # q halves of 512
                        # kt>=12 tiles are only touched in qh2=1 -> bufs=1
                        probT = [ppool.tile([128, 512], F8, tag=f"pT{kt}",
                                            name=f"pT{kt}_{h}_{qh2}",
                                            bufs=(2 if kt < 12 else 1))
                                 for kt in range(12 if qh2 == 0 else NKT)]
                        # batched oblique BD reads: [128, 4, 512] per ktb
                        nktb = 3 if qh2 == 0 else 4
                        bd16s = []
                        for ktb in range(nktb):
                            bd16 = wpool.tile([128, 4, 512], F8, tag="bd16",
                                              bufs=4, name=f"bd16_{qh2}{ktb}")
                            nc.sync.dma_start(out=bd16[:],
                                              in_=obl4(h, qh2, ktb))
                            bd16s.append(bd16)
                        issue_pf(6)
                        bd_tiles = {}
                        for ktb in range(nktb):
                            for qti in range(4):
                                qt = 4 * qh2 + qti
                                kmax = min(qt + 8, NKT - 1)
                                if 4 * ktb > kmax:
                                    continue
                                wdt = min(512, (kmax + 1 - 4 * ktb) * 128)
                                bd = wpool.tile([128, 512], mybir.dt.float32r,
                                                tag="bd",
                                                bufs=7, name=f"bd{qt}_{ktb}")
                                src_sl = bd16s[ktb][:, qti, 0:wdt]
                                r3 = (4 * ktb + qti) % 16
                                if r3 < 9:
                                    nc.gpsimd.tensor_copy(bd[:, 0:wdt], src_sl)
                                elif r3 < 14:
                                    nc.vector.tensor_copy(bd[:, 0:wdt], src_sl)
                                else:
                                    nc.scalar.activation(bd[:, 0:wdt], src_sl,
                                                         AF.Copy)
                                bd_tiles[(qt, ktb)] = bd
                        for kt in range(NKT):
                            qts = [qt for qt in range(4 * qh2, 4 * qh2 + 4)
                                   if qt >= kt - 8]
                            if not qts:
                                continue
                            ps = psA.tile([128, 512], F32, tag="psA", name="pss")
                            nc.tensor.matmul(
                                ps[:], k_h[:, 128 * kt:128 * kt + 128],
                                qh_h[:, 512 * qh2:512 * qh2 + 512],
                                start=True, stop=False)
                            for i, qt in enumerate(qts):
                                bd = bd_tiles[(qt, kt // 4)]
                                bo = 128 * (kt % 4)
                                sub = 128 * (qt - 4 * qh2)
                                FR = mybir.dt.float32r
                                nc.tensor.matmul(ps[:, sub:sub + 128].bitcast(FR),
                                                 bd[:, bo:bo + 128],
                                                 identr[:],
                                                 is_transpose=True,
                                                 start=False,
                                                 stop=(i == len(qts) - 1),
                                                 skip_group_check=True)
                            blo, bhi = qts[0], 4 * qh2 + 4
                            sub = 128 * (blo - 4 * qh2)
                            w = 128 * (bhi - blo)
                            nc.scalar.activation(
                                probT[kt][:, sub:sub + w],
                                ps[:, sub:sub + w], AF.Exp)

                        # vec per query tile in this half
                        for qt in range(4 * qh2, 4 * qh2 + 4):
                            kmax = min(qt + 8, NKT - 1)
                            pv = psV.tile([128, 65], F32, tag="psV", name="pv")
                            sub = 128 * (qt - 4 * qh2)
                            for kt in range(kmax + 1):
                                nc.tensor.matmul(
                                    pv[:], probT[kt][:, sub:sub + 128],
                                    v_tok[kt][:, h, :],
                                    start=(kt == 0), stop=(kt == kmax))
                            rec = wpool.tile([128, 1], F32, tag="rec", name="rec")
                            nc.vector.reciprocal(rec[:], pv[:, 64:65])
                            vn = wpool.tile([128, 64], BF16, tag="vn", name="vn")
                            nc.vector.tensor_scalar_mul(vn[:], pv[:, 0:64],
                                                        rec[:])
                            pt = psT.tile([64, 128], BF16, tag="psT", name="ptr")
                            nc.tensor.matmul(pt[:], vn[:], identb[:],
                                             is_transpose=True,
                                             start=True, stop=True)
                            nc.vector.tensor_copy(
                                vecT_fm[(m, qh2)][p0:p0 + 64,
                                                  128 * (qt % 4):128 * (qt % 4) + 128],
                                pt[:])

                # ---------------- P3: partial attn_out ----------------
                rtb_t = [None, None]
                for g4 in (0, 2, 1, 3):
                    ao = wpool.tile([128, 2, D], BF16, tag="ao", bufs=2,
                                    name="ao")
                    for qti in range(2):
                        qt = 2 * g4 + qti
                        for n in range(D // 512):
                            ps = psA.tile([128, 512], F32, tag="psA", name="pso")
                            for k in range(2):
                                nc.tensor.matmul(
                                    ps[:],
                                    vecT_fm[(k, qt // 4)][:, 128 * (qt % 4):
                                                          128 * (qt % 4) + 128],
                                    woT_t[:, k, 512 * n:512 * n + 512],
                                    start=(k == 0), stop=(k == 1))
                            if n == 0:
                                nc.vector.tensor_copy(
                                    ao[:, qti, 512 * n:512 * n + 512], ps[:])
                            else:
                                nc.scalar.activation(
                                    ao[:, qti, 512 * n:512 * n + 512], ps[:],
                                    AF.Copy)
                    nc.sync.dma_start(
                        out=attn_part[256 * g4:256 * g4 + 256, :].rearrange(
                            "(a b) c -> b a c", a=2, b=128),
                        in_=ao[:])
                    # RS (or its single-sim stand-in read) as soon as the
                    # needed attn_part rows are complete
                    if single_sim and g4 in (0, 2):
                        rtb = wpool.tile([128, D], BF16, tag="rsx", bufs=2,
                                         name=f"rs{g4 // 2}")
                        nc.sync.dma_start(
                            out=rtb[:],
                            in_=attn_part[512 * (g4 // 2):
                                          512 * (g4 // 2) + 128, :])
                        rtb_t[g4 // 2] = rtb
                    if not single_sim and g4 in (1, 3):
                        s = g4 // 2
                        nc.gpsimd.collective_compute(
                            "ReduceScatter", ALU.add,
                            replica_groups=[[0, 1, 2, 3], [4, 5, 6, 7]],
                            ins=[attn_part[512 * s:512 * s + 512, :]],
                            outs=[rs_out[128 * s:128 * s + 128, :]])

            # ================ FF scope ================
            with tc.tile_pool(name="ff", bufs=1) as fpool, \
                 tc.tile_pool(name="w2s", bufs=3) as w2pool:

                gbt0 = fpool.tile([128, D], BF16, tag="gbt0", name="gbt0")
                nc.scalar.dma_start(out=gbt0[:], in_=gbr[0:128, :])

                def ln_stats(x_t, s_acc, act_sq=False):
                    junk = fpool.tile([128, D], BF16, tag="lnjunk", bufs=1,
                                      name="junk")
                    q1 = wpool.tile([128, 1], F32, tag="lnq", name="q1")
                    if act_sq:
                        nc.scalar.activation(junk[:], x_t[:], AF.Square,
                                             accum_out=q1[:])
                    else:
                        nc.vector.scalar_tensor_tensor(
                            out=junk[:], in0=x_t[:], scalar=1.0, in1=x_t[:],
                            op0=ALU.mult, op1=ALU.mult, accum_out=q1[:])
                    mn = wpool.tile([128, 1], F32, tag="lnm", name="mn")
                    nc.vector.tensor_scalar_mul(mn[:], s_acc[:], 1.0 / D)
                    mn2 = wpool.tile([128, 1], F32, tag="lnm2", name="mn2")
                    nc.vector.tensor_scalar(out=mn2[:], in0=mn[:], scalar1=mn[:],
                                            scalar2=1e-5, op0=ALU.mult,
                                            op1=ALU.subtract)
                    # ve = q1/D - mn^2 + 1e-5  (= q1/D - (mn^2 - 1e-5))
                    ve = wpool.tile([128, 1], F32, tag="lnve", name="ve")
                    nc.vector.tensor_scalar(out=ve[:], in0=q1[:], scalar1=1.0 / D,
                                            scalar2=mn2[:], op0=ALU.mult,
                                            op1=ALU.subtract)
                    rc = wpool.tile([128, 1], F32, tag="lnrc", name="rc")
                    nc.vector.reciprocal(rc[:], ve[:])
                    rstd = wpool.tile([128, 1], F32, tag="lnrstd", name="rstd")
                    nc.scalar.activation(rstd[:], rc[:], AF.Sqrt)
                    return mn, rstd

                def fused_ln(x_t, s_acc, out_n, act_sq=False):
                    mn, rstd = ln_stats(x_t, s_acc, act_sq)
                    mb = wpool.tile([128, 1], F32, tag="lnmb", name="mb")
                    nc.vector.tensor_scalar(out=mb[:], in0=mn[:],
                                            scalar1=rstd[:], scalar2=-1.0,
                                            op0=ALU.mult, op1=ALU.mult)
                    nc.scalar.activation(out_n[:], x_t[:], AF.Identity,
                                         scale=rstd[:], bias=mb[:])

                # affine rows tile; DMAs deferred past the LN1 boundary
                gbt1 = fpool.tile([128, 4, D], BF16, tag="gbt1", name="gbt1")

                # FF2 psum groups + b2 injection (PE is idle here)
                hps = {}
                hps[(0, 0)] = psB.tile([128, 512], F32, tag="psB", name="h2ps00")
                hps[(0, 1)] = psB.tile([128, 512], F32, tag="psB", name="h2ps01")
                hps[(1, 0)] = psV.tile([128, 512], F32, tag="psV", name="h2ps10")
                hps[(1, 1)] = psT.tile([128, 512], F32, tag="psT", name="h2ps11")
                for (t, n), hp in hps.items():
                    nc.tensor.matmul(hp[:], onesr[:, 0:128],
                                     gbt0[0:1, 512 * n:512 * n + 512],
                                     start=True, stop=False)

                # P5: residual + LN1
                for t in range(ROWS // 128):
                    if single_sim:
                        rtb = rtb_t[t]
                    else:
                        rtb = fpool.tile([128, D], BF16, tag="rsx", bufs=2,
                                         name=f"rs{t}")
                        nc.sync.dma_start(out=rtb[:],
                                          in_=rs_out[128 * t:128 * t + 128, :])
                    x1 = fpool.tile([128, D], F32, tag="lnx", bufs=2,
                                    name=f"x1_{t}")
                    s1 = wpool.tile([128, 1], F32, tag="lns", name=f"s1_{t}")
                    nc.vector.scalar_tensor_tensor(
                        out=x1[:], in0=wres_t[t][:], scalar=1.0, in1=rtb[:],
                        op0=ALU.mult, op1=ALU.add, accum_out=s1[:])
                    fused_ln(x1, s1, out1n[t])

                # P6: FF — transpose normalized out1 to feature-major
                out1_fm = []
                for k in range(NDC):
                    ofm = fpool.tile([128, ROWS], BF16, tag=f"o1fm{k}",
                                     name=f"o1fm{k}")
                    out1_fm.append(ofm)
                for t in range(ROWS // 128):
                    for k in range(NDC):
                        pt = psA.tile([128, 128], BF16, tag="psA", name="ptf")
                        nc.tensor.matmul(pt[:], out1n[t][:, 128 * k:128 * k + 128],
                                         identb[:], is_transpose=True,
                                         start=True, stop=True)
                        nc.vector.tensor_copy(out1_fm[k][:, 128 * t:128 * t + 128],
                                              pt[:])

                # w2 stream for 8 <= mi < 24: 8 chunks of 2 row-tiles,
                # issued interleaved into the mi loop. The split-region
                # chunks (mi 24..31) get their own fully-resident tag,
                # read by both ff_tail passes.
                w2sb = [w2pool.tile([128, 2, D], BF16, tag="w2", bufs=2,
                                    name=f"w2s{b}") for b in range(8)]
                w2tl = [w2pool.tile([128, 2, D], BF16, tag="w2t", bufs=4,
                                    name=f"w2t{b}") for b in range(4)]
                w2q = iter(range(12))

                def issue_w2(n):
                    for _ in range(n):
                        b = next(w2q, None)
                        if b is None:
                            return
                        dst = w2sb[b][:] if b < 8 else w2tl[b - 8][:]
                        nc.sync.dma_start(
                            out=dst,
                            in_=w2T[1024 + 256 * b:1024 + 256 * b + 256, :]
                            .rearrange("(a b) c -> b a c", a=2, b=128))

                def w2_sl(mi):
                    if mi < 8:
                        return w2pre[mi // 2][:, mi % 2, :]
                    if mi < 24:
                        return w2sb[(mi - 8) // 2][:, (mi - 8) % 2, :]
                    return w2tl[(mi - 24) // 2][:, (mi - 24) % 2, :]

                # FF1 + FF2 interleaved per mi; last SPLIT mi's run per
                # row-tile so t0's LN2 overlaps t1's remaining FF work.
                SPLIT = 24
                issue_w2(2)
                # affine rows (off critical path) + lazy out1 affine
                for i in range(4):
                    nc.scalar.dma_start(
                        out=gbt1[:, i, :],
                        in_=gbr[128 + 128 * i:256 + 128 * i, :])
                for t in range(ROWS // 128):
                    nc.gpsimd.tensor_tensor(out=out1a[t][:], in0=out1n[t][:],
                                            in1=gbt1[:, 0, :], op=ALU.mult)
                    nc.gpsimd.tensor_tensor(out=out1a[t][:], in0=out1a[t][:],
                                            in1=gbt1[:, 1, :], op=ALU.add)
                for mi in range(SPLIT):
                    ps = psA.tile([128, ROWS], F32, tag="psA", name="psh1")
                    for k in range(NDC):
                        nc.tensor.matmul(
                            ps[:], w1_sl(mi, k), out1_fm[k][:],
                            start=(k == 0), stop=(k == NDC - 1))
                    ht = fpool.tile([128, ROWS], BF16, tag="h1T", bufs=4,
                                    name=f"h1T{mi}")
                    nc.scalar.activation(ht[:], ps[:], AF.Relu,
                                         bias=b1c_t[:, mi:mi + 1])
                    w2t = w2_sl(mi)
                    for t in range(ROWS // 128):
                        for n in range(D // 512):
                            nc.tensor.matmul(
                                hps[(t, n)][:], ht[:, 128 * t:128 * t + 128],
                                w2t[:, 512 * n:512 * n + 512],
                                start=False, stop=False)
                    if mi < 10:
                        issue_w2(1)

                def ff_tail(t):
                    for mi in range(SPLIT, NMI):
                        ps = psA.tile([128, 128], F32, tag="psA", name="psh1")
                        for k in range(NDC):
                            nc.tensor.matmul(
                                ps[:], w1_sl(mi, k),
                                out1_fm[k][:, 128 * t:128 * t + 128],
                                start=(k == 0), stop=(k == NDC - 1))
                        ht = fpool.tile([128, 128], BF16, tag="h1Ts", bufs=4,
                                        name=f"h1Ts{mi}_{t}")
                        nc.scalar.activation(ht[:], ps[:], AF.Relu,
                                             bias=b1c_t[:, mi:mi + 1])
                        w2t = w2_sl(mi)
                        for n in range(D // 512):
                            nc.tensor.matmul(
                                hps[(t, n)][:], ht[:],
                                w2t[:, 512 * n:512 * n + 512],
                                start=False, stop=(mi == NMI - 1))

                def ln2_store(t):
                    x2 = fpool.tile([128, D], F32, tag="lnx", bufs=2,
                                    name=f"x2_{t}")
                    s2a = wpool.tile([128, 2], F32, tag="lns2", name=f"s2a_{t}")
                    for n in range(D // 512):
                        nc.vector.scalar_tensor_tensor(
                            out=x2[:, 512 * n:512 * n + 512],
                            in0=hps[(t, n)][:], scalar=1.0,
                            in1=out1a[t][:, 512 * n:512 * n + 512],
                            op0=ALU.mult, op1=ALU.add,
                            accum_out=s2a[:, n:n + 1])
                    s2 = wpool.tile([128, 1], F32, tag="lns", name=f"s2_{t}")
                    nc.vector.tensor_reduce(s2[:], s2a[:],
                                            axis=mybir.AxisListType.X,
                                            op=ALU.add)
                    mn2, rstd2 = ln_stats(x2, s2, act_sq=True)
                    yn = fpool.tile([128, D], F32, tag="yn", bufs=2,
                                    name=f"yn_{t}")
                    for n in range(D // 512):
                        c = slice(512 * n, 512 * n + 512)
                        # u = (x2 - mn) * g2 ; y = u * rstd + b2
                        nc.vector.scalar_tensor_tensor(
                            out=yn[:, c], in0=x2[:, c], scalar=mn2[:],
                            in1=gbt1[:, 2, c], op0=ALU.subtract, op1=ALU.mult)
                        nc.vector.scalar_tensor_tensor(
                            out=yn[:, c], in0=yn[:, c], scalar=rstd2[:],
                            in1=gbt1[:, 3, c], op0=ALU.mult, op1=ALU.add)
                        nc.sync.dma_start(out=y[128 * t:128 * t + 128, c],
                                          in_=yn[:, c])

                ff_tail(0)
                ln2_store(0)      # overlaps ff_tail(1) on PE
                ff_tail(1)
                ln2_store(1)

    nc.compile()
    return nc


def _prep_inputs(w, r, mems, W_qkv, W_r, W_o, r_w_bias, r_r_bias,
                 ln1_g, ln1_b, ff_W1, ff_b1, ff_W2, ff_b2, ln2_g, ln2_b,
                 attn_mask=None):
    import ml_dtypes
    f32 = np.float32
    bf16 = ml_dtypes.bfloat16
    cat = np.concatenate([mems, w], axis=0)            # [KLEN, B, D]
    cat_fm = [np.ascontiguousarray(cat[:, b, :].T).astype(bf16)
              for b in range(BSZ)]
    f8 = ml_dtypes.float8_e4m3
    r_fm = np.ascontiguousarray(r.T).astype(f8)
    # fold LN1 affine into FF1: x@W1^T with x = t1*g1 + b1
    #   -> t1@(W1*g1)^T + (b1@W1^T)
    g1 = np.asarray(ln1_g, f32)
    b1v = np.asarray(ln1_b, f32)
    W1 = np.asarray(ff_W1, f32)
    w1T = np.ascontiguousarray((W1 * g1[None, :]).T).astype(bf16)  # [D, DI]
    b1f = np.asarray(ff_b1, f32) + W1 @ b1v
    w2T = np.ascontiguousarray(np.asarray(ff_W2, f32).T).astype(bf16)
    woT_full = np.ascontiguousarray(W_o.T, dtype=f32)  # [H*DH, D]
    b1c = np.ascontiguousarray(b1f.reshape(NMI, 128).T)  # [128, NMI]
    rows5 = np.stack([np.asarray(ff_b2, f32), g1, b1v,
                      np.asarray(ln2_g, f32),
                      np.asarray(ln2_b, f32)], axis=0)      # [5, D]
    gbr = np.ascontiguousarray(
        np.broadcast_to(rows5[:, None, :], (5, 128, D)).reshape(640, D)
    ).astype(bf16)

    in_maps = []
    for c in range(N_CORES):
        b, g = c // 4, c % 4
        sl = slice(HD_G * g, HD_G * g + HD_G)
        wkT = np.asarray(W_qkv, f32)[H * DH:2 * H * DH][sl].T
        wqT = np.asarray(W_qkv, f32)[0:H * DH][sl].T
        wrT = np.asarray(W_r, f32)[sl].T
        wvT = np.asarray(W_qkv, f32)[2 * H * DH:3 * H * DH][sl].T
        wpk = np.concatenate([wkT, wqT, wrT], axis=1)       # [D, 3*HD_G]
        rwbv = np.asarray(r_w_bias, f32).reshape(-1)[sl]
        rrbv = np.asarray(r_r_bias, f32).reshape(-1)[sl]
        bias = np.stack([
            rwbv[0:128], rwbv[128:256], rrbv[0:128], rrbv[128:256],
            rwbv[0:128] * SCALE, rwbv[128:256] * SCALE,
            rrbv[0:128] * SCALE, rrbv[128:256] * SCALE,
        ], axis=1)                                          # [128, 8]
        m = {
            "cat_fm": cat_fm[b],
            "r_fm": r_fm,
            "wpk": np.ascontiguousarray(wpk).astype(f8),
            "wvk": np.ascontiguousarray(wvT).astype(bf16),
            "biases": np.ascontiguousarray(bias),
            "woT": np.ascontiguousarray(woT_full[sl]).astype(bf16),
            "w1T": w1T, "b1c": b1c, "w2T": w2T,
            "gbr": gbr,
            "wres": np.ascontiguousarray(np.concatenate(
                [np.asarray(w, f32)[128 * g:128 * g + 128, b, :],
                 np.asarray(w, f32)[512 + 128 * g:512 + 128 * g + 128, b, :]],
                axis=0)).astype(bf16),
        }
        in_maps.append(m)
    return in_maps


def kernel(**inputs):
    from concourse.bass_utils import run_bass_kernel_spmd
    nc = _build()
    in_maps = _prep_inputs(**{k: np.asarray(v) for k, v in inputs.items()})
    res = run_bass_kernel_spmd(nc, in_maps, list(range(N_CORES)))
    out = np.empty((QLEN, BSZ, D), np.float32)
    for c in range(N_CORES):
        b, g = c // 4, c % 4
        yv = res.results[c]["y"]
        out[128 * g:128 * g + 128, b, :] = yv[0:128]
        out[512 + 128 * g:512 + 128 * g + 128, b, :] = yv[128:256]
    return out


# revision 52
# speedup vs baseline: 1.2116x; 1.0047x over previous
"""Trainium2 Bass kernel for a Transformer-XL (MemTransformerLM) layer.

Sharding over 8 cores: core c = (b = c//4, head-group g = c%4 of 4 heads).
Each core computes its 4 heads' attention for its batch, a partial
attn_out = vec @ W_o[:, heads].T, then a ReduceScatter(+) over the quad
[[0..3],[4..7]] scatters query rows -> each core does LN1+FF+LN2 on its
256 rows. Host reassembles [1024, 2, 1024].

rel_shift: B = q_tilde @ rk^T is written per head to DRAM (f8e4, row
stride 2176) in one 4-query-tile batch per half; BD[i,j] = B[i, j-i+1023]
is read back with a batched oblique AP (4 query tiles per DMA), converted
f8->f32 on GPSIMD, and PE-transpose-accumulated into the AC^T PSUM group.
Masking comes free from -240 pad columns (exp underflows to 0); softmax
denominator from a ones-column appended to v.

W1 (and the first 8 W2 row-tiles) are prefetched during the attention
phase into the SBUF slots freed by cat/r/pw, so the FF phase starts
immediately after the ReduceScatter. LN1's affine is folded into W1/b1
host-side (exact); the residual copy of out1 gets the affine lazily on
GPSIMD off the critical path. Both LayerNorms use fused one-pass
sum/var accumulation (scalar_tensor_tensor accum_out + Act Square
accum_out) spread across DVE/Act/Pool.
"""
import functools
import numpy as np

QLEN, MLEN, BSZ = 1024, 1024, 2
KLEN = QLEN + MLEN
D, H, DH, DI = 1024, 16, 64, 4096
HPG = 4                      # heads per group (per core)
HD_G = HPG * DH              # 256
N_CORES = 8
SCALE = 1.0 / (DH ** 0.5)
NEG = -1e30
BW = 2176                    # padded DRAM width for B (>= 2175)
NQT = QLEN // 128            # 8 query tiles of 128
NKT = KLEN // 128            # 16 key tiles of 128
NDC = D // 128               # 8 d-chunks
NMI = DI // 128              # 32 inner tiles
ROWS = QLEN // 4             # 256 rows per core after RS


@functools.lru_cache(maxsize=2)
def _build(single_sim=False):
    import concourse.bacc as bacc
    import concourse.mybir as mybir
    import concourse.tile as tile
    from concourse import masks
    import bass_rust

    F32 = mybir.dt.float32
    BF16 = mybir.dt.bfloat16
    F8 = mybir.dt.float8e4
    AF = mybir.ActivationFunctionType
    ALU = mybir.AluOpType

    nc = bacc.Bacc("TRN2", target_bir_lowering=False, debug=False,
                   num_devices=N_CORES)

    def din(name, shape, dt=F32):
        return nc.dram_tensor(name, shape, dt, kind="ExternalInput")

    cat_fm = din("cat_fm", [D, KLEN], BF16)   # [d, mems||w tokens], this b
    r_fm = din("r_fm", [D, KLEN], F8)         # r transposed
    wpk = din("wpk", [D, 3 * HD_G], F8)       # [wkT | wqT | wrT]
    wvk = din("wvk", [D, HD_G], BF16)         # wvT
    biases = din("biases", [128, 8])          # raw rwb/rrb + pre-scaled
    woT = din("woT", [HD_G, D], BF16)         # W_o^T rows for group
    w1T = din("w1T", [D, DI], BF16)           # g1 pre-folded in
    b1c = din("b1c", [128, NMI])              # b1 (+W1@ln1_b) packed col-wise
    w2T = din("w2T", [DI, D], BF16)
    gbr = din("gbr", [5 * 128, D], BF16)      # rows: ffb2, g1, b1, g2, b2
    wres = din("wres", [ROWS, D], BF16)       # w rows for residual

    Bh = [nc.dram_tensor(f"Bh{h}", [QLEN * BW], F8) for h in range(HPG)]
    if single_sim:
        attn_part = nc.dram_tensor("attn_part", [QLEN, D], BF16,
                                   kind="ExternalOutput")
    else:
        attn_part = nc.dram_tensor("attn_part", [QLEN, D], BF16)
    rs_out = nc.dram_tensor("rs_out", [ROWS, D], BF16)
    y = nc.dram_tensor("y", [ROWS, D], F32, kind="ExternalOutput")

    def obl4(h, half, ktb):
        # batched oblique: BD tiles [128 q, 4 qtiles, 512 keys] at
        # (qt = 4*half + qti, kt = 4*ktb)
        off = 1023 + 512 * half * (BW - 1) + 512 * ktb
        return bass_rust.AP(tensor=Bh[h].ap().tensor, offset=off,
                            ap=[[BW - 1, 128], [128 * (BW - 1), 4], [1, 512]])

    def bwrite4(h, half, c0):
        # B row block [128, 4 qtiles, BW-? cols] at rows 512*half, col c0
        off = 512 * half * BW + c0
        return bass_rust.AP(tensor=Bh[h].ap().tensor, offset=off,
                            ap=[[BW, 128], [128 * BW, 4], [1, KLEN - c0]])

    def bpad(h):
        # all pad columns of head h as one flat write
        off = 2048
        return bass_rust.AP(tensor=Bh[h].ap().tensor, offset=off,
                            ap=[[128 * BW, NQT], [BW, 128], [1, BW - 2048]])

    with tile.TileContext(nc) as tc:
        with tc.tile_pool(name="const", bufs=1) as cpool, \
             tc.tile_pool(name="slots", bufs=1) as spool, \
             tc.tile_pool(name="work", bufs=2) as wpool, \
             tc.tile_pool(name="psA", bufs=3, space="PSUM") as psA, \
             tc.tile_pool(name="psB", bufs=3, space="PSUM") as psB, \
             tc.tile_pool(name="psV", bufs=1, space="PSUM") as psV, \
             tc.tile_pool(name="psT", bufs=1, space="PSUM") as psT:

            # ---------------- global constants ----------------
            identb = cpool.tile([128, 128], BF16, tag="identb")
            masks.make_identity(nc, identb[:])
            identr = cpool.tile([128, 128], mybir.dt.float32r, tag="identr")
            nc.scalar.activation(identr[:], identb[:], AF.Copy)
            bias_t = cpool.tile([128, 8], F32, tag="bias")
            nc.scalar.dma_start(out=bias_t[:], in_=biases[:])
            b1c_t = cpool.tile([128, NMI], F32, tag="b1c")
            nc.scalar.dma_start(out=b1c_t[:], in_=b1c[:])
            # out1 lives across scopes: normalized (t1) + affined copies
            out1n = [cpool.tile([128, D], BF16, tag=f"o1n{t}", name=f"o1n{t}")
                     for t in range(ROWS // 128)]
            out1a = [cpool.tile([128, D], BF16, tag=f"o1a{t}", name=f"o1a{t}")
                     for t in range(ROWS // 128)]

            wres_t = [cpool.tile([128, D], BF16, tag=f"wres{t}",
                                 name=f"wres{t}") for t in range(ROWS // 128)]
            onesr = cpool.tile([1, 128], BF16, tag="onesr")
            nc.vector.memset(onesr[:], 1.0)
            # pad cols: large-negative f8 so exp() underflows to 0
            zpad = cpool.tile([128, (BW - 2048) * NQT], F8, tag="zpad")
            nc.vector.memset(zpad[:], -240.0)
            for h in range(HPG):
                nc.scalar.dma_start(out=bpad(h), in_=zpad[:])

            # ------------- reusable big slots (outer pool) -------------
            # catA/rA: 4 tags x [128, 2, 2048] bf16 each (1MB) ->
            # later reused for W1 blocks. pwA: 4 tags x [128, 2, 1024]
            # -> later reused for the first 8 W2 row-tiles.
            catB, rA, pwA, pwV = [], [], [], []
            for k in range(NDC):
                t = spool.tile([128, KLEN], BF16, tag=f"catB{k}",
                               name=f"catB{k}")
                nc.sync.dma_start(out=t[:],
                                  in_=cat_fm[128 * k:128 * k + 128, :])
                catB.append(t)
                if k % 2 == 1:
                    j = k // 2
                    t = spool.tile([128, 2, 3 * HD_G], F8, tag=f"pwA{j}",
                                   name=f"pwA{j}",
                                   padded_shape=[128, 2, 2 * D])
                    nc.sync.dma_start(
                        out=t[:], in_=wpk[256 * j:256 * j + 256, :].rearrange(
                            "(a b) c -> b a c", a=2, b=128))
                    pwA.append(t)
                    t = spool.tile([128, 2, HD_G], BF16, tag=f"pwV{j}",
                                   name=f"pwV{j}")
                    nc.sync.dma_start(
                        out=t[:], in_=wvk[256 * j:256 * j + 256, :].rearrange(
                            "(a b) c -> b a c", a=2, b=128))
                    pwV.append(t)
            for j in range(4):
                t = spool.tile([128, 2, KLEN], F8, tag=f"rA{j}",
                               name=f"rA{j}", padded_shape=[128, 2, 2 * KLEN])
                nc.sync.dma_start(
                    out=t[:], in_=r_fm[256 * j:256 * j + 256, :].rearrange(
                        "(a b) c -> b a c", a=2, b=128))
                rA.append(t)

            def cat_sl(k):
                return catB[k][:]

            def r_sl(k):
                return rA[k // 2][:, k % 2, :]

            def pw_sl(k, which, m):
                base = {"wkT": 0, "wqT": 1, "wrT": 2}[which] * HD_G
                return pwA[k // 2][:, k % 2, base + 128 * m:base + 128 * m + 128]

            def pw_v(k):
                return pwV[k // 2][:, k % 2, :]

            # ================ attention scope ================
            with tc.tile_pool(name="attn", bufs=1) as apool, \
                 tc.tile_pool(name="prob", bufs=2) as ppool:

                woT_t = apool.tile([128, 2, D], BF16, tag="woT", name="woT_t")

                k_fm, rk_fm, qh_fm, qt_fm = [], [], [], []
                for m in range(2):
                    k_fm.append(apool.tile([128, KLEN], BF16, tag=f"kfm{m}",
                                           name=f"kfm{m}"))
                    rk_fm.append(apool.tile([128, KLEN], F8, tag=f"rkfm{m}",
                                            name=f"rkfm{m}"))
                    qh_fm.append(apool.tile([128, QLEN], BF16, tag=f"qhfm{m}",
                                            name=f"qhfm{m}"))
                    qt_fm.append(apool.tile([128, QLEN], F8, tag=f"qtfm{m}",
                                            name=f"qtfm{m}"))
                for m in range(2):
                    for n in range(KLEN // 512):
                        ps = psA.tile([128, 512], F32, tag="psA", name="psk")
                        for k in range(NDC):
                            nc.tensor.matmul(
                                ps[:], pw_sl(k, "wkT", m),
                                cat_sl(k)[:, 512 * n:512 * n + 512],
                                start=(k == 0), stop=(k == NDC - 1))
                        nc.scalar.activation(k_fm[m][:, 512 * n:512 * n + 512],
                                             ps[:], AF.Copy)
                    for n in range(QLEN // 512):
                        ps = psA.tile([128, 512], F32, tag="psA", name="psq")
                        for k in range(NDC):
                            nc.tensor.matmul(
                                ps[:], pw_sl(k, "wqT", m),
                                cat_sl(k)[:, MLEN + 512 * n:MLEN + 512 * n + 512],
                                start=(k == 0), stop=(k == NDC - 1))
                        # (q + bias) * SCALE on DVE, cast to bf16
                        nc.vector.tensor_scalar(
                            out=qh_fm[m][:, 512 * n:512 * n + 512], in0=ps[:],
                            scalar1=bias_t[:, m:m + 1], scalar2=SCALE,
                            op0=ALU.add, op1=ALU.mult)
                        nc.scalar.activation(
                            qt_fm[m][:, 512 * n:512 * n + 512], ps[:],
                            AF.Identity, scale=SCALE,
                            bias=bias_t[:, 6 + m:7 + m])
                # v token-major with interleaved ones cols: [128, 4, 65]
                v_tok = []
                for kt in range(NKT):
                    vt = apool.tile([128, HPG, 65], BF16, tag=f"vtok{kt}",
                                    name=f"vtok{kt}")
                    ps = psB.tile([128, HD_G], F32, tag="psB", name="psv")
                    for k in range(NDC):
                        nc.tensor.matmul(
                            ps[:], cat_sl(k)[:, 128 * kt:128 * kt + 128],
                            pw_v(k), start=(k == 0), stop=(k == NDC - 1))
                    nc.vector.memset(vt[:, :, 64:65], 1.0)
                    nc.scalar.activation(
                        vt[:, :, 0:64],
                        ps[:].rearrange("p (a b) -> p a b", a=HPG, b=64),
                        AF.Copy)
                    v_tok.append(vt)
                # rk projection
                for m in range(2):
                    for n in range(KLEN // 512):
                        ps = psA.tile([128, 512], F32, tag="psA", name="psr")
                        for k in range(NDC):
                            nc.tensor.matmul(
                                ps[:], pw_sl(k, "wrT", m),
                                r_sl(k)[:, 512 * n:512 * n + 512],
                                start=(k == 0), stop=(k == NDC - 1))
                        nc.scalar.activation(rk_fm[m][:, 512 * n:512 * n + 512],
                                             ps[:], AF.Copy)

                # ---- FF weight prefetch tiles (fill freed cat/r/pw slots).
                # DMAs are issued interleaved into the head loop below;
                # each DMA <= 512KB to bound queue head-of-line delay.
                w1blk = [[], []]          # [0][k]: [128,2048]; [1][j]: [128,2,2048]
                for k in range(NDC):
                    w1blk[0].append(spool.tile([128, KLEN], BF16,
                                               tag=f"catB{k}", name=f"w1b0_{k}"))
                for j in range(4):
                    w1blk[1].append(spool.tile([128, 2, KLEN], BF16,
                                               tag=f"rA{j}", name=f"w1b1_{j}"))
                w2pre = []
                for j in range(4):
                    w2pre.append(spool.tile([128, 2, D], BF16, tag=f"pwA{j}",
                                            name=f"w2pre{j}"))
                pf_dmas = []
                for k in range(NDC):
                    pf_dmas.append((w1blk[0][k][:],
                                    w1T[128 * k:128 * k + 128, 0:2048]))
                for j in range(4):
                    for j2 in range(2):
                        pf_dmas.append((
                            w1blk[1][j][:, j2, :],
                            w1T[256 * j + 128 * j2:256 * j + 128 * j2 + 128,
                                2048:4096]))
                for j in range(4):
                    for j2 in range(2):
                        pf_dmas.append((
                            w2pre[j][:, j2, :],
                            w2T[256 * j + 128 * j2:256 * j + 128 * j2 + 128, :]))
                for t in range(ROWS // 128):
                    pf_dmas.append((wres_t[t][:],
                                    wres[128 * t:128 * t + 128, :]))
                pf_dmas.insert(4, (woT_t[:], woT[:].rearrange(
                    "(a b) c -> b a c", a=2, b=128)))

                def w1_sl(mi, k):
                    mo = 128 * (mi % 16)
                    if mi < 16:
                        return w1blk[0][k][:, mo:mo + 128]
                    return w1blk[1][k // 2][:, k % 2, mo:mo + 128]

                # ---------------- P2: attention per head ----------------
                vecT_fm = {}
                for m in range(2):
                    for hf in range(2):
                        vecT_fm[(m, hf)] = apool.tile(
                            [128, QLEN // 2], BF16, tag=f"vecT{m}_{hf}",
                            name=f"vecT{m}_{hf}")
                pf_iter = iter(pf_dmas)

                def issue_pf(n):
                    for _ in range(n):
                        item = next(pf_iter, None)
                        if item is None:
                            return
                        dst, src = item
                        nc.sync.dma_start(out=dst, in_=src)

                for h in range(HPG):
                    m, p0 = h // 2, 64 * (h % 2)
                    qh_h = qh_fm[m][p0:p0 + 64, :]
                    qt_h = qt_fm[m][p0:p0 + 64, :]
                    k_h = k_fm[m][p0:p0 + 64, :]
                    rk_h = rk_fm[m][p0:p0 + 64, :]

                    # B = q_tilde @ rk^T -> DRAM f8 rows, one DMA per half.
                    # For qt<=3 the first 512 cols are never read back.
                    for half in range(2):
                        ct0 = 1 - half
                        bs = wpool.tile([128, 4, KLEN - 512 * ct0], F8,
                                        tag=f"bstage{half}", bufs=1, name="bs",
                                        padded_shape=[128, 4, KLEN - 512 * ct0])
                        for qti in range(4):
                            qt = 4 * half + qti
                            for ct in range(ct0, KLEN // 512):
                                ps = psB.tile([128, 512], F32, tag="psB",
                                              name="psb")
                                nc.tensor.matmul(
                                    ps[:], qt_h[:, 128 * qt:128 * qt + 128],
                                    rk_h[:, 512 * ct:512 * ct + 512],
                                    start=True, stop=True)
                                co = 512 * (ct - ct0)
                                if ct == 1:
                                    nc.scalar.activation(
                                        bs[:, qti, co:co + 512], ps[:], AF.Copy)
                                else:
                                    nc.vector.tensor_copy(
                                        bs[:, qti, co:co + 512], ps[:])
                        nc.sync.dma_start(out=bwrite4(h, half, 512 * ct0),
                                          in_=bs[:])

                    for qh2 in range(2):       # q halves of 512
                        # kt>=12 tiles are only touched in qh2=1 -> bufs=1
                        probT = [ppool.tile([128, 512], F8, tag=f"pT{kt}",
                                            name=f"pT{kt}_{h}_{qh2}",
                                            bufs=(2 if kt < 12 else 1))
                                 for kt in range(12 if qh2 == 0 else NKT)]
                        # batched oblique BD reads: [128, 4, 512] per ktb
                        nktb = 3 if qh2 == 0 else 4
                        bd16s = []
                        for ktb in range(nktb):
                            bd16 = wpool.tile([128, 4, 512], F8, tag="bd16",
                                              bufs=4, name=f"bd16_{qh2}{ktb}")
                            nc.sync.dma_start(out=bd16[:],
                                              in_=obl4(h, qh2, ktb))
                            bd16s.append(bd16)
                        issue_pf(6)
                        bd_tiles = {}
                        for ktb in range(nktb):
                            for qti in range(4):
                                qt = 4 * qh2 + qti
                                kmax = min(qt + 8, NKT - 1)
                                if 4 * ktb > kmax:
                                    continue
                                wdt = min(512, (kmax + 1 - 4 * ktb) * 128)
                                bd = wpool.tile([128, 512], mybir.dt.float32r,
                                                tag="bd",
                                                bufs=7, name=f"bd{qt}_{ktb}")
                                src_sl = bd16s[ktb][:, qti, 0:wdt]
                                r3 = (4 * ktb + qti) % 16
                                if r3 < 9:
                                    nc.gpsimd.tensor_copy(bd[:, 0:wdt], src_sl)
                                elif r3 < 14:
                                    nc.vector.tensor_copy(bd[:, 0:wdt], src_sl)
                                else:
                                    nc.scalar.activation(bd[:, 0:wdt], src_sl,
                                                         AF.Copy)
                                bd_tiles[(qt, ktb)] = bd
                        for kt in range(NKT):
                            qts = [qt for qt in range(4 * qh2, 4 * qh2 + 4)
                                   if qt >= kt - 8]
                            if not qts:
                                continue
                            ps = psA.tile([128, 512], F32, tag="psA", name="pss")
                            nc.tensor.matmul(
                                ps[:], k_h[:, 128 * kt:128 * kt + 128],
                                qh_h[:, 512 * qh2:512 * qh2 + 512],
                                start=True, stop=False)
                            for i, qt in enumerate(qts):
                                bd = bd_tiles[(qt, kt // 4)]
                                bo = 128 * (kt % 4)
                                sub = 128 * (qt - 4 * qh2)
                                FR = mybir.dt.float32r
                                nc.tensor.matmul(ps[:, sub:sub + 128].bitcast(FR),
                                                 bd[:, bo:bo + 128],
                                                 identr[:],
                                                 is_transpose=True,
                                                 start=False,
                                                 stop=(i == len(qts) - 1),
                                                 skip_group_check=True)
                            blo, bhi = qts[0], 4 * qh2 + 4
                            sub = 128 * (blo - 4 * qh2)
                            w = 128 * (bhi - blo)
                            nc.scalar.activation(
                                probT[kt][:, sub:sub + w],
                                ps[:, sub:sub + w], AF.Exp)

                        # vec per query tile in this half
                        for qt in range(4 * qh2, 4 * qh2 + 4):
                            kmax = min(qt + 8, NKT - 1)
                            pv = psV.tile([128, 65], F32, tag="psV", name="pv")
                            sub = 128 * (qt - 4 * qh2)
                            for kt in range(kmax + 1):
                                nc.tensor.matmul(
                                    pv[:], probT[kt][:, sub:sub + 128],
                                    v_tok[kt][:, h, :],
                                    start=(kt == 0), stop=(kt == kmax))
                            rec = wpool.tile([128, 1], F32, tag="rec", name="rec")
                            nc.vector.reciprocal(rec[:], pv[:, 64:65])
                            vn = wpool.tile([128, 64], BF16, tag="vn", name="vn")
                            nc.vector.tensor_scalar_mul(vn[:], pv[:, 0:64],
                                                        rec[:])
                            pt = psT.tile([64, 128], BF16, tag="psT", name="ptr")
                            nc.tensor.matmul(pt[:], vn[:], identb[:],
                                             is_transpose=True,
                                             start=True, stop=True)
                            nc.vector.tensor_copy(
                                vecT_fm[(m, qh2)][p0:p0 + 64,
                                                  128 * (qt % 4):128 * (qt % 4) + 128],
                                pt[:])

                # ---------------- P3: partial attn_out ----------------
                # preload the sqrt act-table while Act is idle
                sqd = wpool.tile([1, 1], F32, tag="sqd", name="sqd")
                nc.scalar.activation(sqd[:], bias_t[0:1, 0:1], AF.Sqrt)
                rtb_t = [None, None]
                for g4 in (0, 2, 1, 3):
                    ao = wpool.tile([128, 2, D], BF16, tag="ao", bufs=2,
                                    name="ao")
                    for qti in range(2):
                        qt = 2 * g4 + qti
                        for n in range(D // 512):
                            ps = psA.tile([128, 512], F32, tag="psA", name="pso")
                            for k in range(2):
                                nc.tensor.matmul(
                                    ps[:],
                                    vecT_fm[(k, qt // 4)][:, 128 * (qt % 4):
                                                          128 * (qt % 4) + 128],
                                    woT_t[:, k, 512 * n:512 * n + 512],
                                    start=(k == 0), stop=(k == 1))
                            if n == 0:
                                nc.vector.tensor_copy(
                                    ao[:, qti, 512 * n:512 * n + 512], ps[:])
                            else:
                                nc.scalar.activation(
                                    ao[:, qti, 512 * n:512 * n + 512], ps[:],
                                    AF.Copy)
                    nc.sync.dma_start(
                        out=attn_part[256 * g4:256 * g4 + 256, :].rearrange(
                            "(a b) c -> b a c", a=2, b=128),
                        in_=ao[:])
                    # RS (or its single-sim stand-in read) as soon as the
                    # needed attn_part rows are complete
                    if single_sim and g4 in (0, 2):
                        rtb = wpool.tile([128, D], BF16, tag="rsx", bufs=2,
                                         name=f"rs{g4 // 2}")
                        nc.sync.dma_start(
                            out=rtb[:],
                            in_=attn_part[512 * (g4 // 2):
                                          512 * (g4 // 2) + 128, :])
                        rtb_t[g4 // 2] = rtb
                    if not single_sim and g4 in (1, 3):
                        s = g4 // 2
                        nc.gpsimd.collective_compute(
                            "ReduceScatter", ALU.add,
                            replica_groups=[[0, 1, 2, 3], [4, 5, 6, 7]],
                            ins=[attn_part[512 * s:512 * s + 512, :]],
                            outs=[rs_out[128 * s:128 * s + 128, :]])

            # ================ FF scope ================
            with tc.tile_pool(name="ff", bufs=1) as fpool, \
                 tc.tile_pool(name="w2s", bufs=3) as w2pool:

                gbt0 = fpool.tile([128, D], BF16, tag="gbt0", name="gbt0")
                nc.scalar.dma_start(out=gbt0[:], in_=gbr[0:128, :])

                def ln_stats(x_t, s_acc, act_sq=False):
                    junk = fpool.tile([128, D], BF16, tag="lnjunk", bufs=1,
                                      name="junk")
                    q1 = wpool.tile([128, 1], F32, tag="lnq", name="q1")
                    if act_sq:
                        nc.scalar.activation(junk[:], x_t[:], AF.Square,
                                             accum_out=q1[:])
                    else:
                        nc.vector.scalar_tensor_tensor(
                            out=junk[:], in0=x_t[:], scalar=1.0, in1=x_t[:],
                            op0=ALU.mult, op1=ALU.mult, accum_out=q1[:])
                    mn = wpool.tile([128, 1], F32, tag="lnm", name="mn")
                    nc.vector.tensor_scalar_mul(mn[:], s_acc[:], 1.0 / D)
                    mn2 = wpool.tile([128, 1], F32, tag="lnm2", name="mn2")
                    nc.vector.tensor_scalar(out=mn2[:], in0=mn[:], scalar1=mn[:],
                                            scalar2=1e-5, op0=ALU.mult,
                                            op1=ALU.subtract)
                    # ve = q1/D - mn^2 + 1e-5  (= q1/D - (mn^2 - 1e-5))
                    ve = wpool.tile([128, 1], F32, tag="lnve", name="ve")
                    nc.vector.tensor_scalar(out=ve[:], in0=q1[:], scalar1=1.0 / D,
                                            scalar2=mn2[:], op0=ALU.mult,
                                            op1=ALU.subtract)
                    rc = wpool.tile([128, 1], F32, tag="lnrc", name="rc")
                    nc.vector.reciprocal(rc[:], ve[:])
                    rstd = wpool.tile([128, 1], F32, tag="lnrstd", name="rstd")
                    nc.scalar.activation(rstd[:], rc[:], AF.Sqrt)
                    return mn, rstd

                def fused_ln(x_t, s_acc, out_n, act_sq=False):
                    mn, rstd = ln_stats(x_t, s_acc, act_sq)
                    mb = wpool.tile([128, 1], F32, tag="lnmb", name="mb")
                    nc.vector.tensor_scalar(out=mb[:], in0=mn[:],
                                            scalar1=rstd[:], scalar2=-1.0,
                                            op0=ALU.mult, op1=ALU.mult)
                    nc.scalar.activation(out_n[:], x_t[:], AF.Identity,
                                         scale=rstd[:], bias=mb[:])

                # affine rows tile; DMAs deferred past the LN1 boundary
                gbt1 = fpool.tile([128, 4, D], BF16, tag="gbt1", name="gbt1")

                # FF2 psum groups + b2 injection (PE is idle here)
                hps = {}
                hps[(0, 0)] = psB.tile([128, 512], F32, tag="psB", name="h2ps00")
                hps[(0, 1)] = psB.tile([128, 512], F32, tag="psB", name="h2ps01")
                hps[(1, 0)] = psV.tile([128, 512], F32, tag="psV", name="h2ps10")
                hps[(1, 1)] = psT.tile([128, 512], F32, tag="psT", name="h2ps11")
                for (t, n), hp in hps.items():
                    nc.tensor.matmul(hp[:], onesr[:, 0:128],
                                     gbt0[0:1, 512 * n:512 * n + 512],
                                     start=True, stop=False)

                # P5: residual + LN1
                for t in range(ROWS // 128):
                    if single_sim:
                        rtb = rtb_t[t]
                    else:
                        rtb = fpool.tile([128, D], BF16, tag="rsx", bufs=2,
                                         name=f"rs{t}")
                        nc.sync.dma_start(out=rtb[:],
                                          in_=rs_out[128 * t:128 * t + 128, :])
                    x1 = fpool.tile([128, D], F32, tag="lnx", bufs=2,
                                    name=f"x1_{t}")
                    s1 = wpool.tile([128, 1], F32, tag="lns", name=f"s1_{t}")
                    nc.vector.scalar_tensor_tensor(
                        out=x1[:], in0=wres_t[t][:], scalar=1.0, in1=rtb[:],
                        op0=ALU.mult, op1=ALU.add, accum_out=s1[:])
                    fused_ln(x1, s1, out1n[t], act_sq=True)

                # P6: FF — transpose normalized out1 to feature-major
                out1_fm = []
                for k in range(NDC):
                    ofm = fpool.tile([128, ROWS], BF16, tag=f"o1fm{k}",
                                     name=f"o1fm{k}")
                    out1_fm.append(ofm)
                for t in range(ROWS // 128):
                    for k in range(NDC):
                        pt = psA.tile([128, 128], BF16, tag="psA", name="ptf")
                        nc.tensor.matmul(pt[:], out1n[t][:, 128 * k:128 * k + 128],
                                         identb[:], is_transpose=True,
                                         start=True, stop=True)
                        nc.vector.tensor_copy(out1_fm[k][:, 128 * t:128 * t + 128],
                                              pt[:])

                # w2 stream for 8 <= mi < 24: 8 chunks of 2 row-tiles,
                # issued interleaved into the mi loop. The split-region
                # chunks (mi 24..31) get their own fully-resident tag,
                # read by both ff_tail passes.
                w2sb = [w2pool.tile([128, 2, D], BF16, tag="w2", bufs=2,
                                    name=f"w2s{b}") for b in range(8)]
                w2tl = [w2pool.tile([128, 2, D], BF16, tag="w2t", bufs=4,
                                    name=f"w2t{b}") for b in range(4)]
                w2q = iter(range(12))

                def issue_w2(n):
                    for _ in range(n):
                        b = next(w2q, None)
                        if b is None:
                            return
                        dst = w2sb[b][:] if b < 8 else w2tl[b - 8][:]
                        nc.sync.dma_start(
                            out=dst,
                            in_=w2T[1024 + 256 * b:1024 + 256 * b + 256, :]
                            .rearrange("(a b) c -> b a c", a=2, b=128))

                def w2_sl(mi):
                    if mi < 8:
                        return w2pre[mi // 2][:, mi % 2, :]
                    if mi < 24:
                        return w2sb[(mi - 8) // 2][:, (mi - 8) % 2, :]
                    return w2tl[(mi - 24) // 2][:, (mi - 24) % 2, :]

                # FF1 + FF2 interleaved per mi; last SPLIT mi's run per
                # row-tile so t0's LN2 overlaps t1's remaining FF work.
                SPLIT = 24
                issue_w2(2)
                # affine rows (off critical path) + lazy out1 affine
                for i in range(4):
                    nc.scalar.dma_start(
                        out=gbt1[:, i, :],
                        in_=gbr[128 + 128 * i:256 + 128 * i, :])
                for t in range(ROWS // 128):
                    nc.gpsimd.tensor_tensor(out=out1a[t][:], in0=out1n[t][:],
                                            in1=gbt1[:, 0, :], op=ALU.mult)
                    nc.gpsimd.tensor_tensor(out=out1a[t][:], in0=out1a[t][:],
                                            in1=gbt1[:, 1, :], op=ALU.add)
                for mi in range(SPLIT):
                    ps = psA.tile([128, ROWS], F32, tag="psA", name="psh1")
                    for k in range(NDC):
                        nc.tensor.matmul(
                            ps[:], w1_sl(mi, k), out1_fm[k][:],
                            start=(k == 0), stop=(k == NDC - 1))
                    ht = fpool.tile([128, ROWS], BF16, tag="h1T", bufs=4,
                                    name=f"h1T{mi}")
                    nc.scalar.activation(ht[:], ps[:], AF.Relu,
                                         bias=b1c_t[:, mi:mi + 1])
                    w2t = w2_sl(mi)
                    for t in range(ROWS // 128):
                        for n in range(D // 512):
                            nc.tensor.matmul(
                                hps[(t, n)][:], ht[:, 128 * t:128 * t + 128],
                                w2t[:, 512 * n:512 * n + 512],
                                start=False, stop=False)
                    if mi < 10:
                        issue_w2(1)

                def ff_tail(t):
                    for mi in range(SPLIT, NMI):
                        ps = psA.tile([128, 128], F32, tag="psA", name="psh1")
                        for k in range(NDC):
                            nc.tensor.matmul(
                                ps[:], w1_sl(mi, k),
                                out1_fm[k][:, 128 * t:128 * t + 128],
                                start=(k == 0), stop=(k == NDC - 1))
                        ht = fpool.tile([128, 128], BF16, tag="h1Ts", bufs=4,
                                        name=f"h1Ts{mi}_{t}")
                        nc.scalar.activation(ht[:], ps[:], AF.Relu,
                                             bias=b1c_t[:, mi:mi + 1])
                        w2t = w2_sl(mi)
                        for n in range(D // 512):
                            nc.tensor.matmul(
                                hps[(t, n)][:], ht[:],
                                w2t[:, 512 * n:512 * n + 512],
                                start=False, stop=(mi == NMI - 1))

                def ln2_store(t):
                    x2 = fpool.tile([128, D], F32, tag="lnx", bufs=2,
                                    name=f"x2_{t}")
                    s2a = wpool.tile([128, 2], F32, tag="lns2", name=f"s2a_{t}")
                    for n in range(D // 512):
                        nc.vector.scalar_tensor_tensor(
                            out=x2[:, 512 * n:512 * n + 512],
                            in0=hps[(t, n)][:], scalar=1.0,
                            in1=out1a[t][:, 512 * n:512 * n + 512],
                            op0=ALU.mult, op1=ALU.add,
                            accum_out=s2a[:, n:n + 1])
                    s2 = wpool.tile([128, 1], F32, tag="lns", name=f"s2_{t}")
                    nc.vector.tensor_reduce(s2[:], s2a[:],
                                            axis=mybir.AxisListType.X,
                                            op=ALU.add)
                    mn2, rstd2 = ln_stats(x2, s2, act_sq=True)
                    yn = fpool.tile([128, D], F32, tag="yn", bufs=2,
                                    name=f"yn_{t}")
                    for n in range(D // 512):
                        c = slice(512 * n, 512 * n + 512)
                        # u = (x2 - mn) * g2 ; y = u * rstd + b2
                        nc.vector.scalar_tensor_tensor(
                            out=yn[:, c], in0=x2[:, c], scalar=mn2[:],
                            in1=gbt1[:, 2, c], op0=ALU.subtract, op1=ALU.mult)
                        nc.vector.scalar_tensor_tensor(
                            out=yn[:, c], in0=yn[:, c], scalar=rstd2[:],
                            in1=gbt1[:, 3, c], op0=ALU.mult, op1=ALU.add)
                        nc.sync.dma_start(out=y[128 * t:128 * t + 128, c],
                                          in_=yn[:, c])

                ff_tail(0)
                ln2_store(0)      # overlaps ff_tail(1) on PE
                ff_tail(1)
                ln2_store(1)

    nc.compile()
    return nc


def _prep_inputs(w, r, mems, W_qkv, W_r, W_o, r_w_bias, r_r_bias,
                 ln1_g, ln1_b, ff_W1, ff_b1, ff_W2, ff_b2, ln2_g, ln2_b,
                 attn_mask=None):
    import ml_dtypes
    f32 = np.float32
    bf16 = ml_dtypes.bfloat16
    cat = np.concatenate([mems, w], axis=0)            # [KLEN, B, D]
    cat_fm = [np.ascontiguousarray(cat[:, b, :].T).astype(bf16)
              for b in range(BSZ)]
    f8 = ml_dtypes.float8_e4m3
    r_fm = np.ascontiguousarray(r.T).astype(f8)
    # fold LN1 affine into FF1: x@W1^T with x = t1*g1 + b1
    #   -> t1@(W1*g1)^T + (b1@W1^T)
    g1 = np.asarray(ln1_g, f32)
    b1v = np.asarray(ln1_b, f32)
    W1 = np.asarray(ff_W1, f32)
    w1T = np.ascontiguousarray((W1 * g1[None, :]).T).astype(bf16)  # [D, DI]
    b1f = np.asarray(ff_b1, f32) + W1 @ b1v
    w2T = np.ascontiguousarray(np.asarray(ff_W2, f32).T).astype(bf16)
    woT_full = np.ascontiguousarray(W_o.T, dtype=f32)  # [H*DH, D]
    b1c = np.ascontiguousarray(b1f.reshape(NMI, 128).T)  # [128, NMI]
    rows5 = np.stack([np.asarray(ff_b2, f32), g1, b1v,
                      np.asarray(ln2_g, f32),
                      np.asarray(ln2_b, f32)], axis=0)      # [5, D]
    gbr = np.ascontiguousarray(
        np.broadcast_to(rows5[:, None, :], (5, 128, D)).reshape(640, D)
    ).astype(bf16)

    in_maps = []
    for c in range(N_CORES):
        b, g = c // 4, c % 4
        sl = slice(HD_G * g, HD_G * g + HD_G)
        wkT = np.asarray(W_qkv, f32)[H * DH:2 * H * DH][sl].T
        wqT = np.asarray(W_qkv, f32)[0:H * DH][sl].T
        wrT = np.asarray(W_r, f32)[sl].T
        wvT = np.asarray(W_qkv, f32)[2 * H * DH:3 * H * DH][sl].T
        wpk = np.concatenate([wkT, wqT, wrT], axis=1)       # [D, 3*HD_G]
        rwbv = np.asarray(r_w_bias, f32).reshape(-1)[sl]
        rrbv = np.asarray(r_r_bias, f32).reshape(-1)[sl]
        bias = np.stack([
            rwbv[0:128], rwbv[128:256], rrbv[0:128], rrbv[128:256],
            rwbv[0:128] * SCALE, rwbv[128:256] * SCALE,
            rrbv[0:128] * SCALE, rrbv[128:256] * SCALE,
        ], axis=1)                                          # [128, 8]
        m = {
            "cat_fm": cat_fm[b],
            "r_fm": r_fm,
            "wpk": np.ascontiguousarray(wpk).astype(f8),
            "wvk": np.ascontiguousarray(wvT).astype(bf16),
            "biases": np.ascontiguousarray(bias),
            "woT": np.ascontiguousarray(woT_full[sl]).astype(bf16),
            "w1T": w1T, "b1c": b1c, "w2T": w2T,
            "gbr": gbr,
            "wres": np.ascontiguousarray(np.concatenate(
                [np.asarray(w, f32)[128 * g:128 * g + 128, b, :],
                 np.asarray(w, f32)[512 + 128 * g:512 + 128 * g + 128, b, :]],
                axis=0)).astype(bf16),
        }
        in_maps.append(m)
    return in_maps


def kernel(**inputs):
    from concourse.bass_utils import run_bass_kernel_spmd
    nc = _build()
    in_maps = _prep_inputs(**{k: np.asarray(v) for k, v in inputs.items()})
    res = run_bass_kernel_spmd(nc, in_maps, list(range(N_CORES)))
    out = np.empty((QLEN, BSZ, D), np.float32)
    for c in range(N_CORES):
        b, g = c // 4, c % 4
        yv = res.results[c]["y"]
        out[128 * g:128 * g + 128, b, :] = yv[0:128]
        out[512 + 128 * g:512 + 128 * g + 128, b, :] = yv[128:256]
    return out


# revision 73
# speedup vs baseline: 1.2380x; 1.0218x over previous
"""Trainium2 Bass kernel for a Transformer-XL (MemTransformerLM) layer.

Sharding over 8 cores: core c = (b = c//4, head-group g = c%4 of 4 heads).
Each core computes its 4 heads' attention for its batch, a partial
attn_out = vec @ W_o[:, heads].T, then a ReduceScatter(+) over the quad
[[0..3],[4..7]] scatters query rows -> each core does LN1+FF+LN2 on its
256 rows. Host reassembles [1024, 2, 1024].

rel_shift: B = q_tilde @ rk^T is written per head to DRAM (f8e4, row
stride 2176) in one 4-query-tile batch per half; BD[i,j] = B[i, j-i+1023]
is read back with a batched oblique AP (4 query tiles per DMA), converted
f8->f32 on GPSIMD, and PE-transpose-accumulated into the AC^T PSUM group.
Masking comes free from -240 pad columns (exp underflows to 0); softmax
denominator from a ones-column appended to v.

W1 (and the first 8 W2 row-tiles) are prefetched during the attention
phase into the SBUF slots freed by cat/r/pw, so the FF phase starts
immediately after the ReduceScatter. LN1's affine is folded into W1/b1
host-side (exact); the residual copy of out1 gets the affine lazily on
GPSIMD off the critical path. Both LayerNorms use fused one-pass
sum/var accumulation (scalar_tensor_tensor accum_out + Act Square
accum_out) spread across DVE/Act/Pool.
"""
import functools
import numpy as np

QLEN, MLEN, BSZ = 1024, 1024, 2
KLEN = QLEN + MLEN
D, H, DH, DI = 1024, 16, 64, 4096
HPG = 4                      # heads per group (per core)
HD_G = HPG * DH              # 256
N_CORES = 8
SCALE = 1.0 / (DH ** 0.5)
NEG = -1e30
BW = 2176                    # padded DRAM width for B (>= 2175)
NQT = QLEN // 128            # 8 query tiles of 128
NKT = KLEN // 128            # 16 key tiles of 128
NDC = D // 128               # 8 d-chunks
NMI = DI // 128              # 32 inner tiles
ROWS = QLEN // 4             # 256 rows per core after RS


@functools.lru_cache(maxsize=2)
def _build(single_sim=False):
    import concourse.bacc as bacc
    import concourse.mybir as mybir
    import concourse.tile as tile
    from concourse import masks
    import bass_rust

    F32 = mybir.dt.float32
    BF16 = mybir.dt.bfloat16
    F8 = mybir.dt.float8e4
    AF = mybir.ActivationFunctionType
    ALU = mybir.AluOpType

    nc = bacc.Bacc("TRN2", target_bir_lowering=False, debug=False,
                   num_devices=N_CORES)

    def din(name, shape, dt=F32):
        return nc.dram_tensor(name, shape, dt, kind="ExternalInput")

    cat_fm = din("cat_fm", [D, KLEN], BF16)   # [d, mems||w tokens], this b
    r_fm = din("r_fm", [D, KLEN], F8)         # r transposed
    wpk = din("wpk", [D, 3 * HD_G], F8)       # [wkT | wqT | wrT]
    wvk = din("wvk", [D, HD_G], BF16)         # wvT
    biases = din("biases", [128, 8])          # raw rwb/rrb + pre-scaled
    woT = din("woT", [HD_G, D], BF16)         # W_o^T rows for group
    w1T = din("w1T", [D, DI], BF16)           # g1 pre-folded in
    b1c = din("b1c", [128, NMI])              # b1 (+W1@ln1_b) packed col-wise
    w2T = din("w2T", [DI, D], BF16)
    gbr = din("gbr", [5 * 128, D], BF16)      # rows: ffb2, g1, b1, g2, b2
    wres = din("wres", [ROWS, D], BF16)       # w rows for residual

    Bh = [nc.dram_tensor(f"Bh{h}", [QLEN * BW], F8) for h in range(HPG)]
    if single_sim:
        attn_part = nc.dram_tensor("attn_part", [QLEN, D], BF16,
                                   kind="ExternalOutput")
    else:
        attn_part = nc.dram_tensor("attn_part", [QLEN, D], BF16)
    rs_out = nc.dram_tensor("rs_out", [ROWS, D], BF16)
    y = nc.dram_tensor("y", [ROWS, D], F32, kind="ExternalOutput")

    def obl4(h, half, ktb):
        # batched oblique: BD tiles [128 q, 4 qtiles, 512 keys] at
        # (qt = 4*half + qti, kt = 4*ktb)
        off = 1023 + 512 * half * (BW - 1) + 512 * ktb
        return bass_rust.AP(tensor=Bh[h].ap().tensor, offset=off,
                            ap=[[BW - 1, 128], [128 * (BW - 1), 4], [1, 512]])

    def bwrite4(h, half, c0):
        # B row block [128, 4 qtiles, BW-? cols] at rows 512*half, col c0
        off = 512 * half * BW + c0
        return bass_rust.AP(tensor=Bh[h].ap().tensor, offset=off,
                            ap=[[BW, 128], [128 * BW, 4], [1, KLEN - c0]])

    def bpad(h):
        # all pad columns of head h as one flat write
        off = 2048
        return bass_rust.AP(tensor=Bh[h].ap().tensor, offset=off,
                            ap=[[128 * BW, NQT], [BW, 128], [1, BW - 2048]])

    with tile.TileContext(nc) as tc:
        with tc.tile_pool(name="const", bufs=1) as cpool, \
             tc.tile_pool(name="slots", bufs=1) as spool, \
             tc.tile_pool(name="work", bufs=2) as wpool, \
             tc.tile_pool(name="psA", bufs=3, space="PSUM") as psA, \
             tc.tile_pool(name="psB", bufs=3, space="PSUM") as psB, \
             tc.tile_pool(name="psV", bufs=1, space="PSUM") as psV, \
             tc.tile_pool(name="psT", bufs=1, space="PSUM") as psT:

            # ---------------- global constants ----------------
            identb = cpool.tile([128, 128], BF16, tag="identb")
            masks.make_identity(nc, identb[:])
            identr = cpool.tile([128, 128], mybir.dt.float32r, tag="identr")
            nc.scalar.activation(identr[:], identb[:], AF.Copy)
            bias_t = cpool.tile([128, 8], F32, tag="bias")
            nc.scalar.dma_start(out=bias_t[:], in_=biases[:])
            b1c_t = cpool.tile([128, NMI], F32, tag="b1c")
            nc.scalar.dma_start(out=b1c_t[:], in_=b1c[:])
            # out1 lives across scopes: normalized (t1) + affined copies
            out1n = [cpool.tile([128, D], BF16, tag=f"o1n{t}", name=f"o1n{t}")
                     for t in range(ROWS // 128)]
            out1a = [cpool.tile([128, D], BF16, tag=f"o1a{t}", name=f"o1a{t}")
                     for t in range(ROWS // 128)]

            wres_t = [cpool.tile([128, D], BF16, tag=f"wres{t}",
                                 name=f"wres{t}") for t in range(ROWS // 128)]
            onesr = cpool.tile([1, 128], BF16, tag="onesr")
            nc.vector.memset(onesr[:], 1.0)
            # pad cols: large-negative f8 so exp() underflows to 0
            zpad = cpool.tile([128, (BW - 2048) * NQT], F8, tag="zpad")
            nc.vector.memset(zpad[:], -240.0)
            for h in range(HPG):
                nc.scalar.dma_start(out=bpad(h), in_=zpad[:])

            # ------------- reusable big slots (outer pool) -------------
            # catA/rA: 4 tags x [128, 2, 2048] bf16 each (1MB) ->
            # later reused for W1 blocks. pwA: 4 tags x [128, 2, 1024]
            # -> later reused for the first 8 W2 row-tiles.
            catB, rA, pwA, pwV = [], [], [], []
            for k in range(NDC):
                t = spool.tile([128, KLEN], BF16, tag=f"catB{k}",
                               name=f"catB{k}")
                nc.sync.dma_start(out=t[:],
                                  in_=cat_fm[128 * k:128 * k + 128, :])
                catB.append(t)
                if k % 2 == 1:
                    j = k // 2
                    t = spool.tile([128, 2, 3 * HD_G], F8, tag=f"pwA{j}",
                                   name=f"pwA{j}",
                                   padded_shape=[128, 2, 2 * D])
                    nc.sync.dma_start(
                        out=t[:], in_=wpk[256 * j:256 * j + 256, :].rearrange(
                            "(a b) c -> b a c", a=2, b=128))
                    pwA.append(t)
                    t = spool.tile([128, 2, HD_G], BF16, tag=f"pwV{j}",
                                   name=f"pwV{j}")
                    nc.sync.dma_start(
                        out=t[:], in_=wvk[256 * j:256 * j + 256, :].rearrange(
                            "(a b) c -> b a c", a=2, b=128))
                    pwV.append(t)
            for j in range(4):
                t = spool.tile([128, 2, KLEN], F8, tag=f"rA{j}",
                               name=f"rA{j}", padded_shape=[128, 2, 2 * KLEN])
                nc.sync.dma_start(
                    out=t[:], in_=r_fm[256 * j:256 * j + 256, :].rearrange(
                        "(a b) c -> b a c", a=2, b=128))
                rA.append(t)

            def cat_sl(k):
                return catB[k][:]

            def r_sl(k):
                return rA[k // 2][:, k % 2, :]

            def pw_sl(k, which, m):
                base = {"wkT": 0, "wqT": 1, "wrT": 2}[which] * HD_G
                return pwA[k // 2][:, k % 2, base + 128 * m:base + 128 * m + 128]

            def pw_v(k):
                return pwV[k // 2][:, k % 2, :]

            # ================ attention scope ================
            with tc.tile_pool(name="attn", bufs=1) as apool, \
                 tc.tile_pool(name="prob", bufs=2) as ppool:

                woT_t = apool.tile([128, 2, D], BF16, tag="woT", name="woT_t")

                k_fm, rk_fm, qh_fm, qt_fm = [], [], [], []
                for m in range(2):
                    k_fm.append(apool.tile([128, KLEN], BF16, tag=f"kfm{m}",
                                           name=f"kfm{m}"))
                    rk_fm.append(apool.tile([128, KLEN], F8, tag=f"rkfm{m}",
                                            name=f"rkfm{m}"))
                    qh_fm.append(apool.tile([128, QLEN], BF16, tag=f"qhfm{m}",
                                            name=f"qhfm{m}"))
                    qt_fm.append(apool.tile([128, QLEN], F8, tag=f"qtfm{m}",
                                            name=f"qtfm{m}"))
                for m in range(2):
                    for n in range(KLEN // 512):
                        ps = psA.tile([128, 512], F32, tag="psA", name="psk")
                        for k in range(NDC):
                            nc.tensor.matmul(
                                ps[:], pw_sl(k, "wkT", m),
                                cat_sl(k)[:, 512 * n:512 * n + 512],
                                start=(k == 0), stop=(k == NDC - 1))
                        nc.scalar.activation(k_fm[m][:, 512 * n:512 * n + 512],
                                             ps[:], AF.Copy)
                    for n in range(QLEN // 512):
                        ps = psA.tile([128, 512], F32, tag="psA", name="psq")
                        for k in range(NDC):
                            nc.tensor.matmul(
                                ps[:], pw_sl(k, "wqT", m),
                                cat_sl(k)[:, MLEN + 512 * n:MLEN + 512 * n + 512],
                                start=(k == 0), stop=(k == NDC - 1))
                        # (q + bias) * SCALE on DVE, cast to bf16
                        nc.vector.tensor_scalar(
                            out=qh_fm[m][:, 512 * n:512 * n + 512], in0=ps[:],
                            scalar1=bias_t[:, m:m + 1], scalar2=SCALE,
                            op0=ALU.add, op1=ALU.mult)
                        nc.scalar.activation(
                            qt_fm[m][:, 512 * n:512 * n + 512], ps[:],
                            AF.Identity, scale=SCALE,
                            bias=bias_t[:, 6 + m:7 + m])
                # v token-major with interleaved ones cols: [128, 4, 65]
                v_tok = []
                for kt in range(NKT):
                    vt = apool.tile([128, HPG, 65], BF16, tag=f"vtok{kt}",
                                    name=f"vtok{kt}")
                    ps = psB.tile([128, HD_G], F32, tag="psB", name="psv")
                    for k in range(NDC):
                        nc.tensor.matmul(
                            ps[:], cat_sl(k)[:, 128 * kt:128 * kt + 128],
                            pw_v(k), start=(k == 0), stop=(k == NDC - 1))
                    nc.vector.memset(vt[:, :, 64:65], 1.0)
                    nc.scalar.activation(
                        vt[:, :, 0:64],
                        ps[:].rearrange("p (a b) -> p a b", a=HPG, b=64),
                        AF.Copy)
                    v_tok.append(vt)
                # rk projection
                for m in range(2):
                    for n in range(KLEN // 512):
                        ps = psA.tile([128, 512], F32, tag="psA", name="psr")
                        for k in range(NDC):
                            nc.tensor.matmul(
                                ps[:], pw_sl(k, "wrT", m),
                                r_sl(k)[:, 512 * n:512 * n + 512],
                                start=(k == 0), stop=(k == NDC - 1))
                        nc.scalar.activation(rk_fm[m][:, 512 * n:512 * n + 512],
                                             ps[:], AF.Copy)

                # ---- FF weight prefetch tiles (fill freed cat/r/pw slots).
                # DMAs are issued interleaved into the head loop below;
                # each DMA <= 512KB to bound queue head-of-line delay.
                w1blk = [[], []]          # [0][k]: [128,2048]; [1][j]: [128,2,2048]
                for k in range(NDC):
                    w1blk[0].append(spool.tile([128, KLEN], BF16,
                                               tag=f"catB{k}", name=f"w1b0_{k}"))
                for j in range(4):
                    w1blk[1].append(spool.tile([128, 2, KLEN], BF16,
                                               tag=f"rA{j}", name=f"w1b1_{j}"))
                w2pre = []
                for j in range(4):
                    w2pre.append(spool.tile([128, 2, D], BF16, tag=f"pwA{j}",
                                            name=f"w2pre{j}"))
                pf_dmas = []
                for k in range(NDC):
                    pf_dmas.append((w1blk[0][k][:],
                                    w1T[128 * k:128 * k + 128, 0:2048]))
                for j in range(4):
                    for j2 in range(2):
                        pf_dmas.append((
                            w1blk[1][j][:, j2, :],
                            w1T[256 * j + 128 * j2:256 * j + 128 * j2 + 128,
                                2048:4096]))
                for j in range(4):
                    for j2 in range(2):
                        pf_dmas.append((
                            w2pre[j][:, j2, :],
                            w2T[256 * j + 128 * j2:256 * j + 128 * j2 + 128, :]))
                for t in range(ROWS // 128):
                    pf_dmas.append((wres_t[t][:],
                                    wres[128 * t:128 * t + 128, :]))
                pf_dmas.insert(4, (woT_t[:], woT[:].rearrange(
                    "(a b) c -> b a c", a=2, b=128)))

                def w1_sl(mi, k):
                    mo = 128 * (mi % 16)
                    if mi < 16:
                        return w1blk[0][k][:, mo:mo + 128]
                    return w1blk[1][k // 2][:, k % 2, mo:mo + 128]

                # ---------------- P2: attention per head ----------------
                vecT_fm = {}
                for m in range(2):
                    for hf in range(2):
                        vecT_fm[(m, hf)] = apool.tile(
                            [128, QLEN // 2], BF16, tag=f"vecT{m}_{hf}",
                            name=f"vecT{m}_{hf}")
                pf_iter = iter(pf_dmas)

                def issue_pf(n):
                    for _ in range(n):
                        item = next(pf_iter, None)
                        if item is None:
                            return
                        dst, src = item
                        nc.sync.dma_start(out=dst, in_=src)

                for h in range(HPG):
                    m, p0 = h // 2, 64 * (h % 2)
                    qh_h = qh_fm[m][p0:p0 + 64, :]
                    qt_h = qt_fm[m][p0:p0 + 64, :]
                    k_h = k_fm[m][p0:p0 + 64, :]
                    rk_h = rk_fm[m][p0:p0 + 64, :]

                    # B = q_tilde @ rk^T -> DRAM f8 rows, one DMA per half.
                    # For qt<=3 the first 512 cols are never read back.
                    for half in range(2):
                        ct0 = 1 - half
                        bs = wpool.tile([128, 4, KLEN - 512 * ct0], F8,
                                        tag=f"bstage{half}", bufs=1, name="bs",
                                        padded_shape=[128, 4, KLEN - 512 * ct0])
                        for qti in range(4):
                            qt = 4 * half + qti
                            for ct in range(ct0, KLEN // 512):
                                ps = psB.tile([128, 512], F32, tag="psB",
                                              name="psb")
                                nc.tensor.matmul(
                                    ps[:], qt_h[:, 128 * qt:128 * qt + 128],
                                    rk_h[:, 512 * ct:512 * ct + 512],
                                    start=True, stop=True)
                                co = 512 * (ct - ct0)
                                if ct == 1:
                                    nc.scalar.activation(
                                        bs[:, qti, co:co + 512], ps[:], AF.Copy)
                                else:
                                    nc.vector.tensor_copy(
                                        bs[:, qti, co:co + 512], ps[:])
                        nc.sync.dma_start(out=bwrite4(h, half, 512 * ct0),
                                          in_=bs[:])

                    for qh2 in range(2):       # q halves of 512
                        # kt>=12 tiles are only touched in qh2=1 -> bufs=1
                        probT = [ppool.tile([128, 512], F8, tag=f"pT{kt}",
                                            name=f"pT{kt}_{h}_{qh2}",
                                            bufs=(2 if kt < 12 else 1))
                                 for kt in range(12 if qh2 == 0 else NKT)]
                        # batched oblique BD reads: [128, 4, 512] per ktb
                        nktb = 3 if qh2 == 0 else 4
                        bd16s = []
                        for ktb in range(nktb):
                            bd16 = wpool.tile([128, 4, 512], F8, tag="bd16",
                                              bufs=4, name=f"bd16_{qh2}{ktb}")
                            nc.sync.dma_start(out=bd16[:],
                                              in_=obl4(h, qh2, ktb))
                            bd16s.append(bd16)
                        issue_pf(6)
                        bd_tiles = {}
                        for ktb in range(nktb):
                            for qti in range(4):
                                qt = 4 * qh2 + qti
                                kmax = min(qt + 8, NKT - 1)
                                if 4 * ktb > kmax:
                                    continue
                                wdt = min(512, (kmax + 1 - 4 * ktb) * 128)
                                bd = wpool.tile([128, 512], mybir.dt.float32r,
                                                tag="bd",
                                                bufs=7, name=f"bd{qt}_{ktb}")
                                src_sl = bd16s[ktb][:, qti, 0:wdt]
                                r3 = (4 * ktb + qti) % 16
                                if r3 < 7:
                                    nc.gpsimd.tensor_copy(bd[:, 0:wdt], src_sl)
                                elif r3 < 16:
                                    nc.vector.tensor_copy(bd[:, 0:wdt], src_sl)
                                else:
                                    nc.scalar.activation(bd[:, 0:wdt], src_sl,
                                                         AF.Copy)
                                bd_tiles[(qt, ktb)] = bd
                        def do_av(qt):
                            # vec for one query tile, emitted as soon as its
                            # last key tile's probs exist
                            kmax = min(qt + 8, NKT - 1)
                            pv = psV.tile([128, 65], F32, tag="psV", name="pv")
                            sub = 128 * (qt - 4 * qh2)
                            for kt2 in range(kmax + 1):
                                nc.tensor.matmul(
                                    pv[:], probT[kt2][:, sub:sub + 128],
                                    v_tok[kt2][:, h, :],
                                    start=(kt2 == 0), stop=(kt2 == kmax))
                            rec = wpool.tile([128, 1], F32, tag="rec", name="rec")
                            nc.vector.reciprocal(rec[:], pv[:, 64:65])
                            vn = wpool.tile([128, 64], BF16, tag="vn", name="vn")
                            nc.vector.tensor_scalar_mul(vn[:], pv[:, 0:64],
                                                        rec[:])
                            pt = psT.tile([64, 128], BF16, tag="psT", name="ptr")
                            nc.tensor.matmul(pt[:], vn[:], identb[:],
                                             is_transpose=True,
                                             start=True, stop=True)
                            nc.vector.tensor_copy(
                                vecT_fm[(m, qh2)][p0:p0 + 64,
                                                  128 * (qt % 4):128 * (qt % 4) + 128],
                                pt[:])

                        for kt in range(NKT):
                            qts = [qt for qt in range(4 * qh2, 4 * qh2 + 4)
                                   if qt >= kt - 8]
                            if not qts:
                                continue
                            ps = psA.tile([128, 512], F32, tag="psA", name="pss")
                            nc.tensor.matmul(
                                ps[:], k_h[:, 128 * kt:128 * kt + 128],
                                qh_h[:, 512 * qh2:512 * qh2 + 512],
                                start=True, stop=False)
                            for i, qt in enumerate(qts):
                                bd = bd_tiles[(qt, kt // 4)]
                                bo = 128 * (kt % 4)
                                sub = 128 * (qt - 4 * qh2)
                                FR = mybir.dt.float32r
                                nc.tensor.matmul(ps[:, sub:sub + 128].bitcast(FR),
                                                 bd[:, bo:bo + 128],
                                                 identr[:],
                                                 is_transpose=True,
                                                 start=False,
                                                 stop=(i == len(qts) - 1),
                                                 skip_group_check=True)
                            blo, bhi = qts[0], 4 * qh2 + 4
                            sub = 128 * (blo - 4 * qh2)
                            w = 128 * (bhi - blo)
                            nc.scalar.activation(
                                probT[kt][:, sub:sub + w],
                                ps[:, sub:sub + w], AF.Exp)

                        for qt in range(4 * qh2, 4 * qh2 + 4):
                            do_av(qt)

                # ---------------- P3: partial attn_out ----------------
                # preload the sqrt act-table while Act is idle
                sqd = wpool.tile([1, 1], F32, tag="sqd", name="sqd")
                nc.scalar.activation(sqd[:], bias_t[0:1, 0:1], AF.Sqrt)
                rtb_t = [None, None]
                for g4 in (0, 2, 1, 3):
                    ao = wpool.tile([128, 2, D], BF16, tag="ao", bufs=2,
                                    name="ao")
                    for qti in range(2):
                        qt = 2 * g4 + qti
                        for n in range(D // 512):
                            ps = psA.tile([128, 512], F32, tag="psA", name="pso")
                            for k in range(2):
                                nc.tensor.matmul(
                                    ps[:],
                                    vecT_fm[(k, qt // 4)][:, 128 * (qt % 4):
                                                          128 * (qt % 4) + 128],
                                    woT_t[:, k, 512 * n:512 * n + 512],
                                    start=(k == 0), stop=(k == 1))
                            if n == 0:
                                nc.vector.tensor_copy(
                                    ao[:, qti, 512 * n:512 * n + 512], ps[:])
                            else:
                                nc.scalar.activation(
                                    ao[:, qti, 512 * n:512 * n + 512], ps[:],
                                    AF.Copy)
                    nc.sync.dma_start(
                        out=attn_part[256 * g4:256 * g4 + 256, :].rearrange(
                            "(a b) c -> b a c", a=2, b=128),
                        in_=ao[:])
                    # RS (or its single-sim stand-in read) as soon as the
                    # needed attn_part rows are complete
                    if single_sim and g4 in (0, 2):
                        rtb = wpool.tile([128, D], BF16, tag="rsx", bufs=2,
                                         name=f"rs{g4 // 2}")
                        nc.sync.dma_start(
                            out=rtb[:],
                            in_=attn_part[512 * (g4 // 2):
                                          512 * (g4 // 2) + 128, :])
                        rtb_t[g4 // 2] = rtb
                    if not single_sim and g4 in (1, 3):
                        s = g4 // 2
                        nc.gpsimd.collective_compute(
                            "ReduceScatter", ALU.add,
                            replica_groups=[[0, 1, 2, 3], [4, 5, 6, 7]],
                            ins=[attn_part[512 * s:512 * s + 512, :]],
                            outs=[rs_out[128 * s:128 * s + 128, :]])

            # ================ FF scope ================
            with tc.tile_pool(name="ff", bufs=1) as fpool, \
                 tc.tile_pool(name="w2s", bufs=3) as w2pool:

                gbt0 = fpool.tile([128, D], BF16, tag="gbt0", name="gbt0")
                nc.scalar.dma_start(out=gbt0[:], in_=gbr[0:128, :])

                def ln_stats(x_t, s_acc, act_sq=False):
                    junk = fpool.tile([128, D], BF16, tag="lnjunk", bufs=1,
                                      name="junk")
                    q1 = wpool.tile([128, 1], F32, tag="lnq", name="q1")
                    if act_sq:
                        nc.scalar.activation(junk[:], x_t[:], AF.Square,
                                             accum_out=q1[:])
                    else:
                        nc.vector.scalar_tensor_tensor(
                            out=junk[:], in0=x_t[:], scalar=1.0, in1=x_t[:],
                            op0=ALU.mult, op1=ALU.mult, accum_out=q1[:])
                    mn = wpool.tile([128, 1], F32, tag="lnm", name="mn")
                    nc.vector.tensor_scalar_mul(mn[:], s_acc[:], 1.0 / D)
                    mn2 = wpool.tile([128, 1], F32, tag="lnm2", name="mn2")
                    nc.vector.tensor_scalar(out=mn2[:], in0=mn[:], scalar1=mn[:],
                                            scalar2=1e-5, op0=ALU.mult,
                                            op1=ALU.subtract)
                    # ve = q1/D - mn^2 + 1e-5  (= q1/D - (mn^2 - 1e-5))
                    ve = wpool.tile([128, 1], F32, tag="lnve", name="ve")
                    nc.vector.tensor_scalar(out=ve[:], in0=q1[:], scalar1=1.0 / D,
                                            scalar2=mn2[:], op0=ALU.mult,
                                            op1=ALU.subtract)
                    rc = wpool.tile([128, 1], F32, tag="lnrc", name="rc")
                    nc.vector.reciprocal(rc[:], ve[:])
                    rstd = wpool.tile([128, 1], F32, tag="lnrstd", name="rstd")
                    nc.scalar.activation(rstd[:], rc[:], AF.Sqrt)
                    return mn, rstd

                def fused_ln(x_t, s_acc, out_n, act_sq=False):
                    mn, rstd = ln_stats(x_t, s_acc, act_sq)
                    mb = wpool.tile([128, 1], F32, tag="lnmb", name="mb")
                    nc.vector.tensor_scalar(out=mb[:], in0=mn[:],
                                            scalar1=rstd[:], scalar2=-1.0,
                                            op0=ALU.mult, op1=ALU.mult)
                    nc.scalar.activation(out_n[:], x_t[:], AF.Identity,
                                         scale=rstd[:], bias=mb[:])

                # affine rows tile; DMAs deferred past the LN1 boundary
                gbt1 = fpool.tile([128, 4, D], BF16, tag="gbt1", name="gbt1")

                # FF2 psum groups + b2 injection (PE is idle here)
                hps = {}
                hps[(0, 0)] = psB.tile([128, 512], F32, tag="psB", name="h2ps00")
                hps[(0, 1)] = psB.tile([128, 512], F32, tag="psB", name="h2ps01")
                hps[(1, 0)] = psV.tile([128, 512], F32, tag="psV", name="h2ps10")
                hps[(1, 1)] = psT.tile([128, 512], F32, tag="psT", name="h2ps11")
                for (t, n), hp in hps.items():
                    nc.tensor.matmul(hp[:], onesr[:, 0:128],
                                     gbt0[0:1, 512 * n:512 * n + 512],
                                     start=True, stop=False)

                # P5: residual + LN1
                for t in range(ROWS // 128):
                    if single_sim:
                        rtb = rtb_t[t]
                    else:
                        rtb = fpool.tile([128, D], BF16, tag="rsx", bufs=2,
                                         name=f"rs{t}")
                        nc.sync.dma_start(out=rtb[:],
                                          in_=rs_out[128 * t:128 * t + 128, :])
                    x1 = fpool.tile([128, D], F32, tag="lnx", bufs=2,
                                    name=f"x1_{t}")
                    s1 = wpool.tile([128, 1], F32, tag="lns", name=f"s1_{t}")
                    nc.vector.scalar_tensor_tensor(
                        out=x1[:], in0=wres_t[t][:], scalar=1.0, in1=rtb[:],
                        op0=ALU.mult, op1=ALU.add, accum_out=s1[:])
                    fused_ln(x1, s1, out1n[t], act_sq=True)

                # P6: FF — transpose normalized out1 to feature-major
                out1_fm = []
                for k in range(NDC):
                    ofm = fpool.tile([128, ROWS], BF16, tag=f"o1fm{k}",
                                     name=f"o1fm{k}")
                    out1_fm.append(ofm)
                for t in range(ROWS // 128):
                    for k in range(NDC):
                        pt = psA.tile([128, 128], BF16, tag="psA", name="ptf")
                        nc.tensor.matmul(pt[:], out1n[t][:, 128 * k:128 * k + 128],
                                         identb[:], is_transpose=True,
                                         start=True, stop=True)
                        nc.vector.tensor_copy(
                            out1_fm[k][:, 128 * t:128 * t + 128], pt[:])

                # w2 stream for 8 <= mi < 24: 8 chunks of 2 row-tiles,
                # issued interleaved into the mi loop. The split-region
                # chunks (mi 24..31) get their own fully-resident tag,
                # read by both ff_tail passes.
                w2sb = [w2pool.tile([128, 2, D], BF16, tag="w2", bufs=2,
                                    name=f"w2s{b}") for b in range(8)]
                w2tl = [w2pool.tile([128, 2, D], BF16, tag="w2t", bufs=4,
                                    name=f"w2t{b}") for b in range(4)]
                w2q = iter(range(12))

                def issue_w2(n):
                    for _ in range(n):
                        b = next(w2q, None)
                        if b is None:
                            return
                        dst = w2sb[b][:] if b < 8 else w2tl[b - 8][:]
                        nc.sync.dma_start(
                            out=dst,
                            in_=w2T[1024 + 256 * b:1024 + 256 * b + 256, :]
                            .rearrange("(a b) c -> b a c", a=2, b=128))

                def w2_sl(mi):
                    if mi < 8:
                        return w2pre[mi // 2][:, mi % 2, :]
                    if mi < 24:
                        return w2sb[(mi - 8) // 2][:, (mi - 8) % 2, :]
                    return w2tl[(mi - 24) // 2][:, (mi - 24) % 2, :]

                # FF1 + FF2 interleaved per mi; last SPLIT mi's run per
                # row-tile so t0's LN2 overlaps t1's remaining FF work.
                SPLIT = 24
                issue_w2(2)
                # affine rows (off critical path) + lazy out1 affine
                for i in range(4):
                    nc.scalar.dma_start(
                        out=gbt1[:, i, :],
                        in_=gbr[128 + 128 * i:256 + 128 * i, :])
                for t in range(ROWS // 128):
                    nc.gpsimd.tensor_tensor(out=out1a[t][:], in0=out1n[t][:],
                                            in1=gbt1[:, 0, :], op=ALU.mult)
                    nc.gpsimd.tensor_tensor(out=out1a[t][:], in0=out1a[t][:],
                                            in1=gbt1[:, 1, :], op=ALU.add)
                for mi in range(SPLIT):
                    ps = psA.tile([128, ROWS], F32, tag="psA", name="psh1")
                    for k in range(NDC):
                        nc.tensor.matmul(
                            ps[:], w1_sl(mi, k), out1_fm[k][:],
                            start=(k == 0), stop=(k == NDC - 1))
                    ht = fpool.tile([128, ROWS], BF16, tag="h1T", bufs=4,
                                    name=f"h1T{mi}")
                    nc.scalar.activation(ht[:], ps[:], AF.Relu,
                                         bias=b1c_t[:, mi:mi + 1])
                    w2t = w2_sl(mi)
                    for t in range(ROWS // 128):
                        for n in range(D // 512):
                            nc.tensor.matmul(
                                hps[(t, n)][:], ht[:, 128 * t:128 * t + 128],
                                w2t[:, 512 * n:512 * n + 512],
                                start=False, stop=False)
                    if mi < 10:
                        issue_w2(1)

                def ff_tail(t):
                    for mi in range(SPLIT, NMI):
                        ps = psA.tile([128, 128], F32, tag="psA", name="psh1")
                        for k in range(NDC):
                            nc.tensor.matmul(
                                ps[:], w1_sl(mi, k),
                                out1_fm[k][:, 128 * t:128 * t + 128],
                                start=(k == 0), stop=(k == NDC - 1))
                        ht = fpool.tile([128, 128], BF16, tag="h1Ts", bufs=4,
                                        name=f"h1Ts{mi}_{t}")
                        nc.scalar.activation(ht[:], ps[:], AF.Relu,
                                             bias=b1c_t[:, mi:mi + 1])
                        w2t = w2_sl(mi)
                        for n in range(D // 512):
                            nc.tensor.matmul(
                                hps[(t, n)][:], ht[:],
                                w2t[:, 512 * n:512 * n + 512],
                                start=False, stop=(mi == NMI - 1))

                def ln2_store(t):
                    x2 = fpool.tile([128, D], F32, tag="lnx", bufs=2,
                                    name=f"x2_{t}")
                    s2a = wpool.tile([128, 2], F32, tag="lns2", name=f"s2a_{t}")
                    for n in range(D // 512):
                        nc.vector.scalar_tensor_tensor(
                            out=x2[:, 512 * n:512 * n + 512],
                            in0=hps[(t, n)][:], scalar=1.0,
                            in1=out1a[t][:, 512 * n:512 * n + 512],
                            op0=ALU.mult, op1=ALU.add,
                            accum_out=s2a[:, n:n + 1])
                    s2 = wpool.tile([128, 1], F32, tag="lns", name=f"s2_{t}")
                    nc.vector.tensor_reduce(s2[:], s2a[:],
                                            axis=mybir.AxisListType.X,
                                            op=ALU.add)
                    mn2, rstd2 = ln_stats(x2, s2, act_sq=True)
                    yn = fpool.tile([128, D], F32, tag="yn", bufs=2,
                                    name=f"yn_{t}")
                    for n in range(D // 512):
                        c = slice(512 * n, 512 * n + 512)
                        # u = (x2 - mn) * g2 ; y = u * rstd + b2
                        nc.vector.scalar_tensor_tensor(
                            out=yn[:, c], in0=x2[:, c], scalar=mn2[:],
                            in1=gbt1[:, 2, c], op0=ALU.subtract, op1=ALU.mult)
                        nc.vector.scalar_tensor_tensor(
                            out=yn[:, c], in0=yn[:, c], scalar=rstd2[:],
                            in1=gbt1[:, 3, c], op0=ALU.mult, op1=ALU.add)
                        nc.sync.dma_start(out=y[128 * t:128 * t + 128, c],
                                          in_=yn[:, c])

                ff_tail(0)
                ln2_store(0)      # overlaps ff_tail(1) on PE
                ff_tail(1)
                ln2_store(1)

    nc.compile()
    return nc


def _prep_inputs(w, r, mems, W_qkv, W_r, W_o, r_w_bias, r_r_bias,
                 ln1_g, ln1_b, ff_W1, ff_b1, ff_W2, ff_b2, ln2_g, ln2_b,
                 attn_mask=None):
    import ml_dtypes
    f32 = np.float32
    bf16 = ml_dtypes.bfloat16
    cat = np.concatenate([mems, w], axis=0)            # [KLEN, B, D]
    cat_fm = [np.ascontiguousarray(cat[:, b, :].T).astype(bf16)
              for b in range(BSZ)]
    f8 = ml_dtypes.float8_e4m3
    r_fm = np.ascontiguousarray(r.T).astype(f8)
    # fold LN1 affine into FF1: x@W1^T with x = t1*g1 + b1
    #   -> t1@(W1*g1)^T + (b1@W1^T)
    g1 = np.asarray(ln1_g, f32)
    b1v = np.asarray(ln1_b, f32)
    W1 = np.asarray(ff_W1, f32)
    w1T = np.ascontiguousarray((W1 * g1[None, :]).T).astype(bf16)  # [D, DI]
    b1f = np.asarray(ff_b1, f32) + W1 @ b1v
    w2T = np.ascontiguousarray(np.asarray(ff_W2, f32).T).astype(bf16)
    woT_full = np.ascontiguousarray(W_o.T, dtype=f32)  # [H*DH, D]
    b1c = np.ascontiguousarray(b1f.reshape(NMI, 128).T)  # [128, NMI]
    rows5 = np.stack([np.asarray(ff_b2, f32), g1, b1v,
                      np.asarray(ln2_g, f32),
                      np.asarray(ln2_b, f32)], axis=0)      # [5, D]
    gbr = np.ascontiguousarray(
        np.broadcast_to(rows5[:, None, :], (5, 128, D)).reshape(640, D)
    ).astype(bf16)

    in_maps = []
    for c in range(N_CORES):
        b, g = c // 4, c % 4
        sl = slice(HD_G * g, HD_G * g + HD_G)
        wkT = np.asarray(W_qkv, f32)[H * DH:2 * H * DH][sl].T
        wqT = np.asarray(W_qkv, f32)[0:H * DH][sl].T
        wrT = np.asarray(W_r, f32)[sl].T
        wvT = np.asarray(W_qkv, f32)[2 * H * DH:3 * H * DH][sl].T
        wpk = np.concatenate([wkT, wqT, wrT], axis=1)       # [D, 3*HD_G]
        rwbv = np.asarray(r_w_bias, f32).reshape(-1)[sl]
        rrbv = np.asarray(r_r_bias, f32).reshape(-1)[sl]
        bias = np.stack([
            rwbv[0:128], rwbv[128:256], rrbv[0:128], rrbv[128:256],
            rwbv[0:128] * SCALE, rwbv[128:256] * SCALE,
            rrbv[0:128] * SCALE, rrbv[128:256] * SCALE,
        ], axis=1)                                          # [128, 8]
        m = {
            "cat_fm": cat_fm[b],
            "r_fm": r_fm,
            "wpk": np.ascontiguousarray(wpk).astype(f8),
            "wvk": np.ascontiguousarray(wvT).astype(bf16),
            "biases": np.ascontiguousarray(bias),
            "woT": np.ascontiguousarray(woT_full[sl]).astype(bf16),
            "w1T": w1T, "b1c": b1c, "w2T": w2T,
            "gbr": gbr,
            "wres": np.ascontiguousarray(np.concatenate(
                [np.asarray(w, f32)[128 * g:128 * g + 128, b, :],
                 np.asarray(w, f32)[512 + 128 * g:512 + 128 * g + 128, b, :]],
                axis=0)).astype(bf16),
        }
        in_maps.append(m)
    return in_maps


def kernel(**inputs):
    from concourse.bass_utils import run_bass_kernel_spmd
    nc = _build()
    in_maps = _prep_inputs(**{k: np.asarray(v) for k, v in inputs.items()})
    res = run_bass_kernel_spmd(nc, in_maps, list(range(N_CORES)))
    out = np.empty((QLEN, BSZ, D), np.float32)
    for c in range(N_CORES):
        b, g = c // 4, c % 4
        yv = res.results[c]["y"]
        out[128 * g:128 * g + 128, b, :] = yv[0:128]
        out[512 + 128 * g:512 + 128 * g + 128, b, :] = yv[128:256]
    return out


# revision 83
# speedup vs baseline: 1.2384x; 1.0003x over previous
"""Trainium2 Bass kernel for a Transformer-XL (MemTransformerLM) layer.

Sharding over 8 cores: core c = (b = c//4, head-group g = c%4 of 4 heads).
Each core computes its 4 heads' attention for its batch, a partial
attn_out = vec @ W_o[:, heads].T, then a ReduceScatter(+) over the quad
[[0..3],[4..7]] scatters query rows -> each core does LN1+FF+LN2 on its
256 rows. Host reassembles [1024, 2, 1024].

rel_shift: B = q_tilde @ rk^T is written per head to DRAM (f8e4, row
stride 2176) in one 4-query-tile batch per half; BD[i,j] = B[i, j-i+1023]
is read back with a batched oblique AP (4 query tiles per DMA), converted
f8->f32 on GPSIMD, and PE-transpose-accumulated into the AC^T PSUM group.
Masking comes free from -240 pad columns (exp underflows to 0); softmax
denominator from a ones-column appended to v.

W1 (and the first 8 W2 row-tiles) are prefetched during the attention
phase into the SBUF slots freed by cat/r/pw, so the FF phase starts
immediately after the ReduceScatter. LN1's affine is folded into W1/b1
host-side (exact); the residual copy of out1 gets the affine lazily on
GPSIMD off the critical path. Both LayerNorms use fused one-pass
sum/var accumulation (scalar_tensor_tensor accum_out + Act Square
accum_out) spread across DVE/Act/Pool.
"""
import functools
import numpy as np

QLEN, MLEN, BSZ = 1024, 1024, 2
KLEN = QLEN + MLEN
D, H, DH, DI = 1024, 16, 64, 4096
HPG = 4                      # heads per group (per core)
HD_G = HPG * DH              # 256
N_CORES = 8
SCALE = 1.0 / (DH ** 0.5)
NEG = -1e30
BW = 2176                    # padded DRAM width for B (>= 2175)
NQT = QLEN // 128            # 8 query tiles of 128
NKT = KLEN // 128            # 16 key tiles of 128
NDC = D // 128               # 8 d-chunks
NMI = DI // 128              # 32 inner tiles
ROWS = QLEN // 4             # 256 rows per core after RS


@functools.lru_cache(maxsize=2)
def _build(single_sim=False):
    import concourse.bacc as bacc
    import concourse.mybir as mybir
    import concourse.tile as tile
    from concourse import masks
    import bass_rust

    F32 = mybir.dt.float32
    BF16 = mybir.dt.bfloat16
    F8 = mybir.dt.float8e4
    AF = mybir.ActivationFunctionType
    ALU = mybir.AluOpType

    nc = bacc.Bacc("TRN2", target_bir_lowering=False, debug=False,
                   num_devices=N_CORES)

    def din(name, shape, dt=F32):
        return nc.dram_tensor(name, shape, dt, kind="ExternalInput")

    cat_fm = din("cat_fm", [D, KLEN], BF16)   # [d, mems||w tokens], this b
    r_fm = din("r_fm", [D, KLEN], F8)         # r transposed
    wpk = din("wpk", [D, 3 * HD_G], F8)       # [wkT | wqT | wrT]
    wvk = din("wvk", [D, HD_G], BF16)         # wvT
    biases = din("biases", [128, 8])          # raw rwb/rrb + pre-scaled
    woT = din("woT", [HD_G, D], BF16)         # W_o^T rows for group
    w1T = din("w1T", [D, DI], BF16)           # g1 pre-folded in
    b1c = din("b1c", [128, NMI])              # b1 (+W1@ln1_b) packed col-wise
    w2T = din("w2T", [DI, D], BF16)
    gbr = din("gbr", [5 * 128, D], BF16)      # rows: ffb2, g1, b1, g2, b2
    wres = din("wres", [ROWS, D], BF16)       # w rows for residual

    Bh = [nc.dram_tensor(f"Bh{h}", [QLEN * BW], F8) for h in range(HPG)]
    if single_sim:
        attn_part = nc.dram_tensor("attn_part", [QLEN, D], BF16,
                                   kind="ExternalOutput")
    else:
        attn_part = nc.dram_tensor("attn_part", [QLEN, D], BF16)
    rs_out = nc.dram_tensor("rs_out", [ROWS, D], BF16)
    y = nc.dram_tensor("y", [ROWS, D], F32, kind="ExternalOutput")

    def obl4(h, half, ktb):
        # batched oblique: BD tiles [128 q, 4 qtiles, 512 keys] at
        # (qt = 4*half + qti, kt = 4*ktb)
        off = 1023 + 512 * half * (BW - 1) + 512 * ktb
        return bass_rust.AP(tensor=Bh[h].ap().tensor, offset=off,
                            ap=[[BW - 1, 128], [128 * (BW - 1), 4], [1, 512]])

    def bwrite4(h, half, c0):
        # B row block [128, 4 qtiles, BW-? cols] at rows 512*half, col c0
        off = 512 * half * BW + c0
        return bass_rust.AP(tensor=Bh[h].ap().tensor, offset=off,
                            ap=[[BW, 128], [128 * BW, 4], [1, KLEN - c0]])

    def bpad(h):
        # all pad columns of head h as one flat write
        off = 2048
        return bass_rust.AP(tensor=Bh[h].ap().tensor, offset=off,
                            ap=[[128 * BW, NQT], [BW, 128], [1, BW - 2048]])

    with tile.TileContext(nc) as tc:
        with tc.tile_pool(name="const", bufs=1) as cpool, \
             tc.tile_pool(name="slots", bufs=1) as spool, \
             tc.tile_pool(name="work", bufs=2) as wpool, \
             tc.tile_pool(name="psA", bufs=3, space="PSUM") as psA, \
             tc.tile_pool(name="psB", bufs=3, space="PSUM") as psB, \
             tc.tile_pool(name="psV", bufs=1, space="PSUM") as psV, \
             tc.tile_pool(name="psT", bufs=1, space="PSUM") as psT:

            # ---------------- global constants ----------------
            identb = cpool.tile([128, 128], BF16, tag="identb")
            masks.make_identity(nc, identb[:])
            identr = cpool.tile([128, 128], mybir.dt.float32r, tag="identr")
            nc.scalar.activation(identr[:], identb[:], AF.Copy)
            bias_t = cpool.tile([128, 8], F32, tag="bias")
            nc.scalar.dma_start(out=bias_t[:], in_=biases[:])
            b1c_t = cpool.tile([128, NMI], F32, tag="b1c")
            nc.scalar.dma_start(out=b1c_t[:], in_=b1c[:])
            # out1 lives across scopes: normalized (t1) + affined copies
            out1n = [cpool.tile([128, D], BF16, tag=f"o1n{t}", name=f"o1n{t}")
                     for t in range(ROWS // 128)]
            out1a = [cpool.tile([128, D], BF16, tag=f"o1a{t}", name=f"o1a{t}")
                     for t in range(ROWS // 128)]

            wres_t = [cpool.tile([128, D], BF16, tag=f"wres{t}",
                                 name=f"wres{t}") for t in range(ROWS // 128)]
            onesr = cpool.tile([1, 128], BF16, tag="onesr")
            nc.vector.memset(onesr[:], 1.0)
            # pad cols: large-negative f8 so exp() underflows to 0
            zpad = cpool.tile([128, (BW - 2048) * NQT], F8, tag="zpad")
            nc.vector.memset(zpad[:], -240.0)
            for h in range(HPG):
                nc.scalar.dma_start(out=bpad(h), in_=zpad[:])

            # ------------- reusable big slots (outer pool) -------------
            # catA/rA: 4 tags x [128, 2, 2048] bf16 each (1MB) ->
            # later reused for W1 blocks. pwA: 4 tags x [128, 2, 1024]
            # -> later reused for the first 8 W2 row-tiles.
            catB, rA, pwA, pwV = [], [], [], []
            for k in range(NDC):
                t = spool.tile([128, KLEN], BF16, tag=f"catB{k}",
                               name=f"catB{k}")
                nc.sync.dma_start(out=t[:],
                                  in_=cat_fm[128 * k:128 * k + 128, :])
                catB.append(t)
                if k % 2 == 1:
                    j = k // 2
                    t = spool.tile([128, 2, 3 * HD_G], F8, tag=f"pwA{j}",
                                   name=f"pwA{j}",
                                   padded_shape=[128, 2, 2 * D])
                    nc.sync.dma_start(
                        out=t[:], in_=wpk[256 * j:256 * j + 256, :].rearrange(
                            "(a b) c -> b a c", a=2, b=128))
                    pwA.append(t)
                    t = spool.tile([128, 2, HD_G], BF16, tag=f"pwV{j}",
                                   name=f"pwV{j}")
                    nc.sync.dma_start(
                        out=t[:], in_=wvk[256 * j:256 * j + 256, :].rearrange(
                            "(a b) c -> b a c", a=2, b=128))
                    pwV.append(t)
            for j in range(4):
                t = spool.tile([128, 2, KLEN], F8, tag=f"rA{j}",
                               name=f"rA{j}", padded_shape=[128, 2, 2 * KLEN])
                nc.sync.dma_start(
                    out=t[:], in_=r_fm[256 * j:256 * j + 256, :].rearrange(
                        "(a b) c -> b a c", a=2, b=128))
                rA.append(t)

            def cat_sl(k):
                return catB[k][:]

            def r_sl(k):
                return rA[k // 2][:, k % 2, :]

            def pw_sl(k, which, m):
                base = {"wkT": 0, "wqT": 1, "wrT": 2}[which] * HD_G
                return pwA[k // 2][:, k % 2, base + 128 * m:base + 128 * m + 128]

            def pw_v(k):
                return pwV[k // 2][:, k % 2, :]

            # ================ attention scope ================
            with tc.tile_pool(name="attn", bufs=1) as apool, \
                 tc.tile_pool(name="prob", bufs=2) as ppool:

                woT_t = apool.tile([128, 2, D], BF16, tag="woT", name="woT_t")

                k_fm, rk_fm, qh_fm, qt_fm = [], [], [], []
                for m in range(2):
                    k_fm.append(apool.tile([128, KLEN], BF16, tag=f"kfm{m}",
                                           name=f"kfm{m}"))
                    rk_fm.append(apool.tile([128, KLEN], F8, tag=f"rkfm{m}",
                                            name=f"rkfm{m}"))
                    qh_fm.append(apool.tile([128, QLEN], BF16, tag=f"qhfm{m}",
                                            name=f"qhfm{m}"))
                    qt_fm.append(apool.tile([128, QLEN], F8, tag=f"qtfm{m}",
                                            name=f"qtfm{m}"))
                for m in range(2):
                    for n in range(KLEN // 512):
                        ps = psA.tile([128, 512], F32, tag="psA", name="psk")
                        for k in range(NDC):
                            nc.tensor.matmul(
                                ps[:], pw_sl(k, "wkT", m),
                                cat_sl(k)[:, 512 * n:512 * n + 512],
                                start=(k == 0), stop=(k == NDC - 1))
                        nc.scalar.activation(k_fm[m][:, 512 * n:512 * n + 512],
                                             ps[:], AF.Copy)
                    for n in range(QLEN // 512):
                        ps = psA.tile([128, 512], F32, tag="psA", name="psq")
                        for k in range(NDC):
                            nc.tensor.matmul(
                                ps[:], pw_sl(k, "wqT", m),
                                cat_sl(k)[:, MLEN + 512 * n:MLEN + 512 * n + 512],
                                start=(k == 0), stop=(k == NDC - 1))
                        # (q + bias) * SCALE on DVE, cast to bf16
                        nc.vector.tensor_scalar(
                            out=qh_fm[m][:, 512 * n:512 * n + 512], in0=ps[:],
                            scalar1=bias_t[:, m:m + 1], scalar2=SCALE,
                            op0=ALU.add, op1=ALU.mult)
                        nc.scalar.activation(
                            qt_fm[m][:, 512 * n:512 * n + 512], ps[:],
                            AF.Identity, scale=SCALE,
                            bias=bias_t[:, 6 + m:7 + m])
                # v token-major with interleaved ones cols: [128, 4, 65]
                v_tok = []
                for kt in range(NKT):
                    vt = apool.tile([128, HPG, 65], BF16, tag=f"vtok{kt}",
                                    name=f"vtok{kt}")
                    ps = psB.tile([128, HD_G], F32, tag="psB", name="psv")
                    for k in range(NDC):
                        nc.tensor.matmul(
                            ps[:], cat_sl(k)[:, 128 * kt:128 * kt + 128],
                            pw_v(k), start=(k == 0), stop=(k == NDC - 1))
                    nc.vector.memset(vt[:, :, 64:65], 1.0)
                    nc.scalar.activation(
                        vt[:, :, 0:64],
                        ps[:].rearrange("p (a b) -> p a b", a=HPG, b=64),
                        AF.Copy)
                    v_tok.append(vt)
                # rk projection
                for m in range(2):
                    for n in range(KLEN // 512):
                        ps = psA.tile([128, 512], F32, tag="psA", name="psr")
                        for k in range(NDC):
                            nc.tensor.matmul(
                                ps[:], pw_sl(k, "wrT", m),
                                r_sl(k)[:, 512 * n:512 * n + 512],
                                start=(k == 0), stop=(k == NDC - 1))
                        nc.scalar.activation(rk_fm[m][:, 512 * n:512 * n + 512],
                                             ps[:], AF.Copy)

                # ---- FF weight prefetch tiles (fill freed cat/r/pw slots).
                # DMAs are issued interleaved into the head loop below;
                # each DMA <= 512KB to bound queue head-of-line delay.
                w1blk = [[], []]          # [0][k]: [128,2048]; [1][j]: [128,2,2048]
                for k in range(NDC):
                    w1blk[0].append(spool.tile([128, KLEN], BF16,
                                               tag=f"catB{k}", name=f"w1b0_{k}"))
                for j in range(4):
                    w1blk[1].append(spool.tile([128, 2, KLEN], BF16,
                                               tag=f"rA{j}", name=f"w1b1_{j}"))
                w2pre = []
                for j in range(4):
                    w2pre.append(spool.tile([128, 2, D], BF16, tag=f"pwA{j}",
                                            name=f"w2pre{j}"))
                pf_dmas = []
                for k in range(NDC):
                    pf_dmas.append((w1blk[0][k][:],
                                    w1T[128 * k:128 * k + 128, 0:2048]))
                for j in range(4):
                    for j2 in range(2):
                        pf_dmas.append((
                            w1blk[1][j][:, j2, :],
                            w1T[256 * j + 128 * j2:256 * j + 128 * j2 + 128,
                                2048:4096]))
                for j in range(4):
                    for j2 in range(2):
                        pf_dmas.append((
                            w2pre[j][:, j2, :],
                            w2T[256 * j + 128 * j2:256 * j + 128 * j2 + 128, :]))
                for t in range(ROWS // 128):
                    pf_dmas.append((wres_t[t][:],
                                    wres[128 * t:128 * t + 128, :]))
                pf_dmas.insert(4, (woT_t[:], woT[:].rearrange(
                    "(a b) c -> b a c", a=2, b=128)))

                def w1_sl(mi, k):
                    mo = 128 * (mi % 16)
                    if mi < 16:
                        return w1blk[0][k][:, mo:mo + 128]
                    return w1blk[1][k // 2][:, k % 2, mo:mo + 128]

                # ---------------- P2: attention per head ----------------
                vecT_fm = {}
                for m in range(2):
                    for hf in range(2):
                        vecT_fm[(m, hf)] = apool.tile(
                            [128, QLEN // 2], BF16, tag=f"vecT{m}_{hf}",
                            name=f"vecT{m}_{hf}")
                pf_iter = iter(pf_dmas)

                def issue_pf(n):
                    for _ in range(n):
                        item = next(pf_iter, None)
                        if item is None:
                            return
                        dst, src = item
                        nc.sync.dma_start(out=dst, in_=src)

                for h in range(HPG):
                    m, p0 = h // 2, 64 * (h % 2)
                    qh_h = qh_fm[m][p0:p0 + 64, :]
                    qt_h = qt_fm[m][p0:p0 + 64, :]
                    k_h = k_fm[m][p0:p0 + 64, :]
                    rk_h = rk_fm[m][p0:p0 + 64, :]

                    # B = q_tilde @ rk^T -> DRAM f8 rows, one DMA per half.
                    # For qt<=3 the first 512 cols are never read back.
                    for half in range(2):
                        ct0 = 1 - half
                        bs = wpool.tile([128, 4, KLEN - 512 * ct0], F8,
                                        tag=f"bstage{half}", bufs=1, name="bs",
                                        padded_shape=[128, 4, KLEN - 512 * ct0])
                        for qti in range(4):
                            qt = 4 * half + qti
                            for ct in range(ct0, KLEN // 512):
                                ps = psB.tile([128, 512], F32, tag="psB",
                                              name="psb")
                                nc.tensor.matmul(
                                    ps[:], qt_h[:, 128 * qt:128 * qt + 128],
                                    rk_h[:, 512 * ct:512 * ct + 512],
                                    start=True, stop=True)
                                co = 512 * (ct - ct0)
                                if ct == 1:
                                    nc.scalar.activation(
                                        bs[:, qti, co:co + 512], ps[:], AF.Copy)
                                else:
                                    nc.vector.tensor_copy(
                                        bs[:, qti, co:co + 512], ps[:])
                        nc.sync.dma_start(out=bwrite4(h, half, 512 * ct0),
                                          in_=bs[:])

                    for qh2 in range(2):       # q halves of 512
                        # kt>=12 tiles are only touched in qh2=1 -> bufs=1
                        probT = [ppool.tile([128, 512], F8, tag=f"pT{kt}",
                                            name=f"pT{kt}_{h}_{qh2}",
                                            bufs=(2 if kt < 12 else 1))
                                 for kt in range(12 if qh2 == 0 else NKT)]
                        # batched oblique BD reads: [128, 4, 512] per ktb
                        nktb = 3 if qh2 == 0 else 4
                        bd16s = []
                        for ktb in range(nktb):
                            bd16 = wpool.tile([128, 4, 512], F8, tag="bd16",
                                              bufs=4, name=f"bd16_{qh2}{ktb}")
                            nc.sync.dma_start(out=bd16[:],
                                              in_=obl4(h, qh2, ktb))
                            bd16s.append(bd16)
                        issue_pf(6)
                        bd_tiles = {}
                        for ktb in range(nktb):
                            for qti in range(4):
                                qt = 4 * qh2 + qti
                                kmax = min(qt + 8, NKT - 1)
                                if 4 * ktb > kmax:
                                    continue
                                wdt = min(512, (kmax + 1 - 4 * ktb) * 128)
                                bd = wpool.tile([128, 512], mybir.dt.float32r,
                                                tag="bd",
                                                bufs=7, name=f"bd{qt}_{ktb}")
                                src_sl = bd16s[ktb][:, qti, 0:wdt]
                                r3 = (4 * ktb + qti) % 16
                                if r3 < 7:
                                    nc.gpsimd.tensor_copy(bd[:, 0:wdt], src_sl)
                                elif r3 < 16:
                                    nc.vector.tensor_copy(bd[:, 0:wdt], src_sl)
                                else:
                                    nc.scalar.activation(bd[:, 0:wdt], src_sl,
                                                         AF.Copy)
                                bd_tiles[(qt, ktb)] = bd
                        def do_av(qt):
                            # vec for one query tile, emitted as soon as its
                            # last key tile's probs exist
                            kmax = min(qt + 8, NKT - 1)
                            pv = psV.tile([128, 65], F32, tag="psV", name="pv")
                            sub = 128 * (qt - 4 * qh2)
                            for kt2 in range(kmax + 1):
                                nc.tensor.matmul(
                                    pv[:], probT[kt2][:, sub:sub + 128],
                                    v_tok[kt2][:, h, :],
                                    start=(kt2 == 0), stop=(kt2 == kmax))
                            rec = wpool.tile([128, 1], F32, tag="rec", name="rec")
                            nc.vector.reciprocal(rec[:], pv[:, 64:65])
                            vn = wpool.tile([128, 64], BF16, tag="vn", name="vn")
                            nc.vector.tensor_scalar_mul(vn[:], pv[:, 0:64],
                                                        rec[:])
                            pt = psT.tile([64, 128], BF16, tag="psT", name="ptr")
                            nc.tensor.matmul(pt[:], vn[:], identb[:],
                                             is_transpose=True,
                                             start=True, stop=True)
                            nc.vector.tensor_copy(
                                vecT_fm[(m, qh2)][p0:p0 + 64,
                                                  128 * (qt % 4):128 * (qt % 4) + 128],
                                pt[:])

                        for kt in range(NKT):
                            qts = [qt for qt in range(4 * qh2, 4 * qh2 + 4)
                                   if qt >= kt - 8]
                            if not qts:
                                continue
                            ps = psA.tile([128, 512], F32, tag="psA", name="pss")
                            nc.tensor.matmul(
                                ps[:], k_h[:, 128 * kt:128 * kt + 128],
                                qh_h[:, 512 * qh2:512 * qh2 + 512],
                                start=True, stop=False)
                            for i, qt in enumerate(qts):
                                bd = bd_tiles[(qt, kt // 4)]
                                bo = 128 * (kt % 4)
                                sub = 128 * (qt - 4 * qh2)
                                FR = mybir.dt.float32r
                                nc.tensor.matmul(ps[:, sub:sub + 128].bitcast(FR),
                                                 bd[:, bo:bo + 128],
                                                 identr[:],
                                                 is_transpose=True,
                                                 start=False,
                                                 stop=(i == len(qts) - 1),
                                                 skip_group_check=True)
                            blo, bhi = qts[0], 4 * qh2 + 4
                            sub = 128 * (blo - 4 * qh2)
                            w = 128 * (bhi - blo)
                            nc.scalar.activation(
                                probT[kt][:, sub:sub + w],
                                ps[:, sub:sub + w], AF.Exp)

                        for qt in range(4 * qh2, 4 * qh2 + 4):
                            do_av(qt)

                # ---------------- P3: partial attn_out ----------------
                # preload the sqrt act-table while Act is idle
                sqd = wpool.tile([1, 1], F32, tag="sqd", name="sqd")
                nc.scalar.activation(sqd[:], bias_t[0:1, 0:1], AF.Sqrt)
                rtb_t = [None, None]
                for g4 in (0, 2, 1, 3):
                    ao = wpool.tile([128, 2, D], BF16, tag="ao", bufs=2,
                                    name="ao")
                    for qti in range(2):
                        qt = 2 * g4 + qti
                        for n in range(D // 512):
                            ps = psA.tile([128, 512], F32, tag="psA", name="pso")
                            for k in range(2):
                                nc.tensor.matmul(
                                    ps[:],
                                    vecT_fm[(k, qt // 4)][:, 128 * (qt % 4):
                                                          128 * (qt % 4) + 128],
                                    woT_t[:, k, 512 * n:512 * n + 512],
                                    start=(k == 0), stop=(k == 1))
                            if n == 0:
                                nc.vector.tensor_copy(
                                    ao[:, qti, 512 * n:512 * n + 512], ps[:])
                            else:
                                nc.scalar.activation(
                                    ao[:, qti, 512 * n:512 * n + 512], ps[:],
                                    AF.Copy)
                    nc.sync.dma_start(
                        out=attn_part[256 * g4:256 * g4 + 256, :].rearrange(
                            "(a b) c -> b a c", a=2, b=128),
                        in_=ao[:])
                    # RS (or its single-sim stand-in read) as soon as the
                    # needed attn_part rows are complete
                    if single_sim and g4 in (0, 2):
                        rtb = wpool.tile([128, D], BF16, tag="rsx", bufs=2,
                                         name=f"rs{g4 // 2}")
                        nc.sync.dma_start(
                            out=rtb[:],
                            in_=attn_part[512 * (g4 // 2):
                                          512 * (g4 // 2) + 128, :])
                        rtb_t[g4 // 2] = rtb
                    if not single_sim and g4 in (1, 3):
                        s = g4 // 2
                        nc.gpsimd.collective_compute(
                            "ReduceScatter", ALU.add,
                            replica_groups=[[0, 1, 2, 3], [4, 5, 6, 7]],
                            ins=[attn_part[512 * s:512 * s + 512, :]],
                            outs=[rs_out[128 * s:128 * s + 128, :]])

            # ================ FF scope ================
            with tc.tile_pool(name="ff", bufs=1) as fpool, \
                 tc.tile_pool(name="w2s", bufs=3) as w2pool:

                gbt0 = fpool.tile([128, D], BF16, tag="gbt0", name="gbt0")
                nc.scalar.dma_start(out=gbt0[:], in_=gbr[0:128, :])

                def ln_stats(x_t, s_acc, act_sq=False):
                    junk = fpool.tile([128, D], BF16, tag="lnjunk", bufs=1,
                                      name="junk")
                    q1 = wpool.tile([128, 1], F32, tag="lnq", name="q1")
                    if act_sq:
                        nc.scalar.activation(junk[:], x_t[:], AF.Square,
                                             accum_out=q1[:])
                    else:
                        nc.vector.scalar_tensor_tensor(
                            out=junk[:], in0=x_t[:], scalar=1.0, in1=x_t[:],
                            op0=ALU.mult, op1=ALU.mult, accum_out=q1[:])
                    mn = wpool.tile([128, 1], F32, tag="lnm", name="mn")
                    nc.vector.tensor_scalar_mul(mn[:], s_acc[:], 1.0 / D)
                    mn2 = wpool.tile([128, 1], F32, tag="lnm2", name="mn2")
                    nc.vector.tensor_scalar(out=mn2[:], in0=mn[:], scalar1=mn[:],
                                            scalar2=1e-5, op0=ALU.mult,
                                            op1=ALU.subtract)
                    # ve = q1/D - mn^2 + 1e-5  (= q1/D - (mn^2 - 1e-5))
                    ve = wpool.tile([128, 1], F32, tag="lnve", name="ve")
                    nc.vector.tensor_scalar(out=ve[:], in0=q1[:], scalar1=1.0 / D,
                                            scalar2=mn2[:], op0=ALU.mult,
                                            op1=ALU.subtract)
                    rc = wpool.tile([128, 1], F32, tag="lnrc", name="rc")
                    nc.vector.reciprocal(rc[:], ve[:])
                    rstd = wpool.tile([128, 1], F32, tag="lnrstd", name="rstd")
                    nc.scalar.activation(rstd[:], rc[:], AF.Sqrt)
                    return mn, rstd

                def fused_ln(x_t, s_acc, out_n, act_sq=False):
                    mn, rstd = ln_stats(x_t, s_acc, act_sq)
                    mb = wpool.tile([128, 1], F32, tag="lnmb", name="mb")
                    nc.vector.tensor_scalar(out=mb[:], in0=mn[:],
                                            scalar1=rstd[:], scalar2=-1.0,
                                            op0=ALU.mult, op1=ALU.mult)
                    nc.scalar.activation(out_n[:], x_t[:], AF.Identity,
                                         scale=rstd[:], bias=mb[:])

                # affine rows tile; DMAs deferred past the LN1 boundary
                gbt1 = fpool.tile([128, 4, D], BF16, tag="gbt1", name="gbt1")

                # FF2 psum groups + b2 injection (PE is idle here)
                hps = {}
                hps[(0, 0)] = psB.tile([128, 512], F32, tag="psB", name="h2ps00")
                hps[(0, 1)] = psB.tile([128, 512], F32, tag="psB", name="h2ps01")
                hps[(1, 0)] = psV.tile([128, 512], F32, tag="psV", name="h2ps10")
                hps[(1, 1)] = psT.tile([128, 512], F32, tag="psT", name="h2ps11")
                for (t, n), hp in hps.items():
                    nc.tensor.matmul(hp[:], onesr[:, 0:128],
                                     gbt0[0:1, 512 * n:512 * n + 512],
                                     start=True, stop=False)

                # P5: residual + LN1
                for t in range(ROWS // 128):
                    if single_sim:
                        rtb = rtb_t[t]
                    else:
                        rtb = fpool.tile([128, D], BF16, tag="rsx", bufs=2,
                                         name=f"rs{t}")
                        nc.sync.dma_start(out=rtb[:],
                                          in_=rs_out[128 * t:128 * t + 128, :])
                    x1 = fpool.tile([128, D], F32, tag="lnx", bufs=2,
                                    name=f"x1_{t}")
                    s1 = wpool.tile([128, 1], F32, tag="lns", name=f"s1_{t}")
                    nc.vector.scalar_tensor_tensor(
                        out=x1[:], in0=wres_t[t][:], scalar=1.0, in1=rtb[:],
                        op0=ALU.mult, op1=ALU.add, accum_out=s1[:])
                    fused_ln(x1, s1, out1n[t], act_sq=True)

                # P6: FF — transpose normalized out1 to feature-major
                out1_fm = []
                for k in range(NDC):
                    ofm = fpool.tile([128, ROWS], BF16, tag=f"o1fm{k}",
                                     name=f"o1fm{k}")
                    out1_fm.append(ofm)
                for t in range(ROWS // 128):
                    for k in range(NDC):
                        pt = psA.tile([128, 128], BF16, tag="psA", name="ptf")
                        nc.tensor.matmul(pt[:], out1n[t][:, 128 * k:128 * k + 128],
                                         identb[:], is_transpose=True,
                                         start=True, stop=True)
                        nc.vector.tensor_copy(
                            out1_fm[k][:, 128 * t:128 * t + 128], pt[:])

                # w2 stream for 8 <= mi < 24: 8 chunks of 2 row-tiles,
                # issued interleaved into the mi loop. The split-region
                # chunks (mi 24..31) get their own fully-resident tag,
                # read by both ff_tail passes.
                w2sb = [w2pool.tile([128, 2, D], BF16, tag="w2", bufs=2,
                                    name=f"w2s{b}") for b in range(8)]
                w2tl = [w2pool.tile([128, 2, D], BF16, tag="w2t", bufs=4,
                                    name=f"w2t{b}") for b in range(4)]
                w2q = iter(range(12))

                def issue_w2(n):
                    for _ in range(n):
                        b = next(w2q, None)
                        if b is None:
                            return
                        dst = w2sb[b][:] if b < 8 else w2tl[b - 8][:]
                        nc.sync.dma_start(
                            out=dst,
                            in_=w2T[1024 + 256 * b:1024 + 256 * b + 256, :]
                            .rearrange("(a b) c -> b a c", a=2, b=128))

                def w2_sl(mi):
                    if mi < 8:
                        return w2pre[mi // 2][:, mi % 2, :]
                    if mi < 24:
                        return w2sb[(mi - 8) // 2][:, (mi - 8) % 2, :]
                    return w2tl[(mi - 24) // 2][:, (mi - 24) % 2, :]

                # FF1 + FF2 interleaved per mi; last SPLIT mi's run per
                # row-tile so t0's LN2 overlaps t1's remaining FF work.
                SPLIT = 24
                issue_w2(2)
                # affine rows (off critical path) + lazy out1 affine
                for i in range(4):
                    nc.scalar.dma_start(
                        out=gbt1[:, i, :],
                        in_=gbr[128 + 128 * i:256 + 128 * i, :])
                for t in range(ROWS // 128):
                    nc.gpsimd.tensor_tensor(out=out1a[t][:], in0=out1n[t][:],
                                            in1=gbt1[:, 0, :], op=ALU.mult)
                    nc.gpsimd.tensor_tensor(out=out1a[t][:], in0=out1a[t][:],
                                            in1=gbt1[:, 1, :], op=ALU.add)
                for mi in range(SPLIT):
                    ps = psA.tile([128, ROWS], F32, tag="psA", name="psh1")
                    for k in range(NDC):
                        nc.tensor.matmul(
                            ps[:], w1_sl(mi, k), out1_fm[k][:],
                            start=(k == 0), stop=(k == NDC - 1))
                    ht = fpool.tile([128, ROWS], BF16, tag="h1T", bufs=4,
                                    name=f"h1T{mi}")
                    nc.scalar.activation(ht[:], ps[:], AF.Relu,
                                         bias=b1c_t[:, mi:mi + 1])
                    w2t = w2_sl(mi)
                    for t in range(ROWS // 128):
                        for n in range(D // 512):
                            nc.tensor.matmul(
                                hps[(t, n)][:], ht[:, 128 * t:128 * t + 128],
                                w2t[:, 512 * n:512 * n + 512],
                                start=False, stop=False)
                    if mi < 10:
                        issue_w2(1)

                def ff_tail(t):
                    for mi in range(SPLIT, NMI):
                        ps = psA.tile([128, 128], F32, tag="psA", name="psh1")
                        for k in range(NDC):
                            nc.tensor.matmul(
                                ps[:], w1_sl(mi, k),
                                out1_fm[k][:, 128 * t:128 * t + 128],
                                start=(k == 0), stop=(k == NDC - 1))
                        ht = fpool.tile([128, 128], BF16, tag="h1Ts", bufs=4,
                                        name=f"h1Ts{mi}_{t}")
                        nc.scalar.activation(ht[:], ps[:], AF.Relu,
                                             bias=b1c_t[:, mi:mi + 1])
                        w2t = w2_sl(mi)
                        for n in range(D // 512):
                            nc.tensor.matmul(
                                hps[(t, n)][:], ht[:],
                                w2t[:, 512 * n:512 * n + 512],
                                start=False, stop=(mi == NMI - 1))

                def ln2_store(t):
                    x2 = fpool.tile([128, D], F32, tag="lnx", bufs=2,
                                    name=f"x2_{t}")
                    s2a = wpool.tile([128, 2], F32, tag="lns2", name=f"s2a_{t}")
                    for n in range(D // 512):
                        nc.vector.scalar_tensor_tensor(
                            out=x2[:, 512 * n:512 * n + 512],
                            in0=hps[(t, n)][:], scalar=1.0,
                            in1=out1a[t][:, 512 * n:512 * n + 512],
                            op0=ALU.mult, op1=ALU.add,
                            accum_out=s2a[:, n:n + 1])
                    s2 = wpool.tile([128, 1], F32, tag="lns", name=f"s2_{t}")
                    nc.vector.tensor_reduce(s2[:], s2a[:],
                                            axis=mybir.AxisListType.X,
                                            op=ALU.add)
                    mn2, rstd2 = ln_stats(x2, s2, act_sq=True)
                    yn = fpool.tile([128, D], F32, tag="yn", bufs=2,
                                    name=f"yn_{t}")
                    for n in range(D // 512):
                        c = slice(512 * n, 512 * n + 512)
                        # u = (x2 - mn) * g2 ; y = u * rstd + b2
                        nc.vector.scalar_tensor_tensor(
                            out=yn[:, c], in0=x2[:, c], scalar=mn2[:],
                            in1=gbt1[:, 2, c], op0=ALU.subtract, op1=ALU.mult)
                        nc.vector.scalar_tensor_tensor(
                            out=yn[:, c], in0=yn[:, c], scalar=rstd2[:],
                            in1=gbt1[:, 3, c], op0=ALU.mult, op1=ALU.add)
                        nc.sync.dma_start(out=y[128 * t:128 * t + 128, c],
                                          in_=yn[:, c])

                ff_tail(1)
                ln2_store(1)      # overlaps ff_tail(0) on PE
                ff_tail(0)
                ln2_store(0)

    nc.compile()
    return nc


def _prep_inputs(w, r, mems, W_qkv, W_r, W_o, r_w_bias, r_r_bias,
                 ln1_g, ln1_b, ff_W1, ff_b1, ff_W2, ff_b2, ln2_g, ln2_b,
                 attn_mask=None):
    import ml_dtypes
    f32 = np.float32
    bf16 = ml_dtypes.bfloat16
    cat = np.concatenate([mems, w], axis=0)            # [KLEN, B, D]
    cat_fm = [np.ascontiguousarray(cat[:, b, :].T).astype(bf16)
              for b in range(BSZ)]
    f8 = ml_dtypes.float8_e4m3
    r_fm = np.ascontiguousarray(r.T).astype(f8)
    # fold LN1 affine into FF1: x@W1^T with x = t1*g1 + b1
    #   -> t1@(W1*g1)^T + (b1@W1^T)
    g1 = np.asarray(ln1_g, f32)
    b1v = np.asarray(ln1_b, f32)
    W1 = np.asarray(ff_W1, f32)
    w1T = np.ascontiguousarray((W1 * g1[None, :]).T).astype(bf16)  # [D, DI]
    b1f = np.asarray(ff_b1, f32) + W1 @ b1v
    w2T = np.ascontiguousarray(np.asarray(ff_W2, f32).T).astype(bf16)
    woT_full = np.ascontiguousarray(W_o.T, dtype=f32)  # [H*DH, D]
    b1c = np.ascontiguousarray(b1f.reshape(NMI, 128).T)  # [128, NMI]
    rows5 = np.stack([np.asarray(ff_b2, f32), g1, b1v,
                      np.asarray(ln2_g, f32),
                      np.asarray(ln2_b, f32)], axis=0)      # [5, D]
    gbr = np.ascontiguousarray(
        np.broadcast_to(rows5[:, None, :], (5, 128, D)).reshape(640, D)
    ).astype(bf16)

    in_maps = []
    for c in range(N_CORES):
        b, g = c // 4, c % 4
        sl = slice(HD_G * g, HD_G * g + HD_G)
        wkT = np.asarray(W_qkv, f32)[H * DH:2 * H * DH][sl].T
        wqT = np.asarray(W_qkv, f32)[0:H * DH][sl].T
        wrT = np.asarray(W_r, f32)[sl].T
        wvT = np.asarray(W_qkv, f32)[2 * H * DH:3 * H * DH][sl].T
        wpk = np.concatenate([wkT, wqT, wrT], axis=1)       # [D, 3*HD_G]
        rwbv = np.asarray(r_w_bias, f32).reshape(-1)[sl]
        rrbv = np.asarray(r_r_bias, f32).reshape(-1)[sl]
        bias = np.stack([
            rwbv[0:128], rwbv[128:256], rrbv[0:128], rrbv[128:256],
            rwbv[0:128] * SCALE, rwbv[128:256] * SCALE,
            rrbv[0:128] * SCALE, rrbv[128:256] * SCALE,
        ], axis=1)                                          # [128, 8]
        m = {
            "cat_fm": cat_fm[b],
            "r_fm": r_fm,
            "wpk": np.ascontiguousarray(wpk).astype(f8),
            "wvk": np.ascontiguousarray(wvT).astype(bf16),
            "biases": np.ascontiguousarray(bias),
            "woT": np.ascontiguousarray(woT_full[sl]).astype(bf16),
            "w1T": w1T, "b1c": b1c, "w2T": w2T,
            "gbr": gbr,
            "wres": np.ascontiguousarray(np.concatenate(
                [np.asarray(w, f32)[128 * g:128 * g + 128, b, :],
                 np.asarray(w, f32)[512 + 128 * g:512 + 128 * g + 128, b, :]],
                axis=0)).astype(bf16),
        }
        in_maps.append(m)
    return in_maps


def kernel(**inputs):
    from concourse.bass_utils import run_bass_kernel_spmd
    nc = _build()
    in_maps = _prep_inputs(**{k: np.asarray(v) for k, v in inputs.items()})
    res = run_bass_kernel_spmd(nc, in_maps, list(range(N_CORES)))
    out = np.empty((QLEN, BSZ, D), np.float32)
    for c in range(N_CORES):
        b, g = c // 4, c % 4
        yv = res.results[c]["y"]
        out[128 * g:128 * g + 128, b, :] = yv[0:128]
        out[512 + 128 * g:512 + 128 * g + 128, b, :] = yv[128:256]
    return out
